# revision 1
# baseline (speedup 1.0000x reference)
"""GAT 2-layer encoder kernel for Trainium2 (8 NeuronCores, Bass/Tile).

Strategy (graph/data parallel, dst-sharded):
  - Nodes are sharded contiguously across 8 cores (6250 nodes each); each core
    owns the edges whose *destination* lands in its shard (plus self loops).
  - Per layer, each core computes a "node table" for its shard:
        row n = [ h(n) (256 f32) | al_src(n) (4) | al_dst(n) (4) | pad to 320 ]
    with h = x @ W, al_* = per-head dots folded into the matmul weights.
    Tables are AllGather'd so every core has the full [N, 320] table in HBM.
  - Edge phase, per 128-destination window: gather the table rows of all edge
    sources with dma_gather (int16 indices force a lo/hi table split at 32768),
    gather al_dst rows from the local shard table keyed by local dst index,
    build the one-hot (dst == lane) mask with is_equal against an iota row,
    compute ex = exp(leaky_relu(al_src + al_dst)) per edge, scale the gathered
    h rows by ex in-place (ex itself is kept as 4 extra columns), and
    segment-sum with PE matmuls: mask[e,d]^T @ [ex*h | ex] accumulated in PSUM.
    Normalize with a per-partition reciprocal scale fused into a Relu
    activation.
  - Layer 1 epilogue additionally transposes the activations and computes the
    layer-2 table rows; layer 2 epilogue writes the final output rows.

The edge structure (indices, window sizes) is baked into the program as
compile-time constants; per-window chunk counts are maxed across cores so the
same program (SPMD) runs on all 8 cores with per-core index *data*.
"""

import math
import sys

import numpy as np

sys.path.insert(0, "/opt/trn_rl_repo")

P = 128  # partitions


class Cfg:
    def __init__(self, n_nodes=50000, in_dim=128, heads=4, hid=64,
                 n_cores=8, lo_split=32768):
        self.n_nodes = n_nodes
        self.in_dim = in_dim
        self.heads = heads
        self.hid = hid
        self.n_cores = n_cores
        self.d1 = heads * hid                       # 256
        self.ts = ((self.d1 + 8 + 63) // 64) * 64   # table row stride (f32), 320
        self.lo_split = lo_split                    # int16-safe table split
        assert n_nodes % n_cores == 0
        self.shard = n_nodes // n_cores             # 6250
        self.nw = math.ceil(self.shard / P)         # windows per core (49)
        self.shard_pad = self.nw * P


def _plan_edges(cfg, edge_index):
    """Host-side: per-core, per-window padded edge lists in gather layout.

    Returns (plan, per_core_data):
      plan: dict with static (shared across cores) lists nch_lo, nch_hi
      per_core: list of dicts with gidx/alidx [128, NCOLS] int16 and
                dstoff [128, NCHTOT] f32
    """
    NC, SH, NW = cfg.n_cores, cfg.shard, cfg.nw
    src = np.asarray(edge_index[0], dtype=np.int64)
    dst = np.asarray(edge_index[1], dtype=np.int64)
    loops = np.arange(cfg.n_nodes, dtype=np.int64)
    src = np.concatenate([src, loops])
    dst = np.concatenate([dst, loops])

    core = dst // SH
    win = (dst - core * SH) // P

    # bucket edges by (core, window)
    order = np.lexsort((src, win, core))
    src_s, dst_s, core_s, win_s = src[order], dst[order], core[order], win[order]
    key = core_s * NW + win_s
    # boundaries of each (core, window) group
    starts = np.searchsorted(key, np.arange(NC * NW))
    ends = np.searchsorted(key, np.arange(NC * NW) + 1)

    lo_edges = [[None] * NW for _ in range(NC)]
    hi_edges = [[None] * NW for _ in range(NC)]
    for c in range(NC):
        for w in range(NW):
            s, e = starts[c * NW + w], ends[c * NW + w]
            es, ed = src_s[s:e], dst_s[s:e]
            lo = es < cfg.lo_split
            lo_edges[c][w] = (es[lo], ed[lo])
            hi_edges[c][w] = (es[~lo], ed[~lo])

    nch_lo = [0] * NW
    nch_hi = [0] * NW
    for w in range(NW):
        ml = max(len(lo_edges[c][w][0]) for c in range(NC))
        mh = max(len(hi_edges[c][w][0]) for c in range(NC))
        nch_lo[w] = math.ceil(ml / P) if ml else 0
        nch_hi[w] = math.ceil(mh / P) if mh else 0
        if nch_lo[w] == 0 and nch_hi[w] == 0:
            nch_lo[w] = 1  # degenerate empty window: keep shapes legal

    nch = [nch_lo[w] + nch_hi[w] for w in range(NW)]
    nch_tot = sum(nch)
    ncols = 8 * nch_tot  # idx cols per core: (nch*128)/16

    def wrap16(vals, n_idx):
        """[n_idx] int -> [128, n_idx//16] int16 in dma_gather layout."""
        cols = n_idx // 16
        out = np.zeros((16, cols), dtype=np.int16)
        v = np.asarray(vals, dtype=np.int64)
        out[np.arange(n_idx) % 16, np.arange(n_idx) // 16] = v
        return np.tile(out, (8, 1))

    per_core = []
    for c in range(NC):
        gidx = np.zeros((P, ncols), dtype=np.int16)
        alidx = np.zeros((P, ncols), dtype=np.int16)
        dstoff = np.full((P, nch_tot), 255.0, dtype=np.float32)
        gcol = 0
        ccol = 0
        for w in range(NW):
            offs = []
            for (es, ed), nchunks, base in (
                (lo_edges[c][w], nch_lo[w], 0),
                (hi_edges[c][w], nch_hi[w], cfg.lo_split),
            ):
                if nchunks == 0:
                    continue
                n_idx = nchunks * P
                g = np.zeros(n_idx, dtype=np.int64)
                a = np.zeros(n_idx, dtype=np.int64)
                o = np.full(n_idx, 255.0, dtype=np.float32)
                k = len(es)
                g[:k] = es - base
                d_local = ed - c * SH
                a[:k] = d_local
                o[:k] = (d_local - w * P).astype(np.float32)
                gidx[:, gcol:gcol + 8 * nchunks] = wrap16(g, n_idx)
                alidx[:, gcol:gcol + 8 * nchunks] = wrap16(a, n_idx)
                offs.append(o)
                gcol += 8 * nchunks
            o = np.concatenate(offs)
            nck = len(o) // P
            dstoff[:, ccol:ccol + nck] = o.reshape(nck, P).T
            ccol += nck
        assert gcol == ncols and ccol == nch_tot
        per_core.append(dict(gidx=gidx, alidx=alidx, dstoff=dstoff))

    plan = dict(nch_lo=nch_lo, nch_hi=nch_hi, nch=nch, nch_tot=nch_tot,
                ncols=ncols)
    return plan, per_core


def _pack_wext(cfg, W, a_src, a_dst):
    """[K, 256] weight -> [K, 320]: [W | W@Asrc | W@Adst | 0]."""
    K = W.shape[0]
    H, C = cfg.heads, cfg.hid
    out = np.zeros((K, cfg.ts), dtype=np.float32)
    out[:, :cfg.d1] = W
    for h in range(H):
        out[:, cfg.d1 + h] = W[:, h * C:(h + 1) * C] @ a_src[h]
        out[:, cfg.d1 + 4 + h] = W[:, h * C:(h + 1) * C] @ a_dst[h]
    return out


def _ap(t, offset_elems, free_pattern):
    """SBUF AP with explicit free [step, count] dims on top of a tile AP."""
    import concourse.bass as bass
    return bass.AP(t.tensor, t.offset + offset_elems,
                   [list(t.ap[0])] + [list(p) for p in free_pattern])


def _apd(t, offset_elems, pattern):
    """DRAM AP with fully explicit [step, count] dims (no partition dim)."""
    import concourse.bass as bass
    return bass.AP(t.tensor, t.offset + offset_elems,
                   [list(p) for p in pattern])


def build_program(cfg, plan):
    import concourse.bass as bass
    import concourse.mybir as mybir
    import concourse.tile as tile
    from concourse import bacc
    from concourse.masks import make_identity
    from contextlib import ExitStack

    f32 = mybir.dt.float32
    i16 = mybir.dt.int16
    TS, D1, H, C = cfg.ts, cfg.d1, cfg.heads, cfg.hid
    SH, NW, NC = cfg.shard, cfg.nw, cfg.n_cores
    NCH, NCOLS = plan["nch"], plan["ncols"]
    LO = cfg.lo_split
    N = cfg.n_nodes
    HI_ROWS = N - LO
    kin_tiles = cfg.in_dim // P   # 1 for layer 1
    k2_tiles = D1 // P            # 2 for layer 2

    nc = bacc.Bacc()

    xT = nc.dram_tensor("xT", [cfg.in_dim, SH], f32, kind="ExternalInput")
    w1e = nc.dram_tensor("w1e", [cfg.in_dim, TS], f32, kind="ExternalInput")
    w2e = nc.dram_tensor("w2e", [D1, TS], f32, kind="ExternalInput")
    gidx_d = nc.dram_tensor("gidx", [P, NCOLS], i16, kind="ExternalInput")
    alidx_d = nc.dram_tensor("alidx", [P, NCOLS], i16, kind="ExternalInput")
    dstoff_d = nc.dram_tensor("dstoff", [P, plan["nch_tot"]], f32,
                              kind="ExternalInput")
    iotaf_d = nc.dram_tensor("iotaf", [P, P], f32, kind="ExternalInput")
    out_d = nc.dram_tensor("out", [SH, D1], f32, kind="ExternalOutput")

    with ExitStack() as ctx:
        tc = ctx.enter_context(tile.TileContext(nc))
        const = ctx.enter_context(tc.tile_pool(name="const", bufs=1))
        sb = ctx.enter_context(tc.tile_pool(name="sb", bufs=2))
        eps = ctx.enter_context(tc.tile_pool(name="eps", bufs=2))
        psum = ctx.enter_context(tc.tile_pool(name="psum", bufs=2, space="PSUM"))
        dram = ctx.enter_context(tc.tile_pool(name="dram", bufs=1, space="DRAM"))

        # ---- constants / static inputs into SBUF
        w1e_sb = const.tile([cfg.in_dim, TS], f32)
        nc.sync.dma_start(out=w1e_sb[:], in_=w1e[:, :])
        w2e_sb = [const.tile([P, TS], f32, tag=f"w2e{k}", name=f"w2e_sb{k}")
                  for k in range(k2_tiles)]
        for k in range(k2_tiles):
            nc.sync.dma_start(out=w2e_sb[k][:], in_=w2e[k * P:(k + 1) * P, :])
        gidx_sb = const.tile([P, NCOLS], i16)
        nc.sync.dma_start(out=gidx_sb[:], in_=gidx_d[:, :])
        alidx_sb = const.tile([P, NCOLS], i16)
        nc.sync.dma_start(out=alidx_sb[:], in_=alidx_d[:, :])
        dstoff_sb = const.tile([P, plan["nch_tot"]], f32)
        nc.sync.dma_start(out=dstoff_sb[:], in_=dstoff_d[:, :])
        iotaf_sb = const.tile([P, P], f32)
        nc.sync.dma_start(out=iotaf_sb[:], in_=iotaf_d[:, :])
        ident = const.tile([P, P], f32)
        make_identity(nc, ident[:])

        t_shard = [dram.tile([SH, TS], f32, tag=f"tsh{i}", name=f"t_shard{i}")
                   for i in range(2)]
        t_full = [dram.tile([N, TS], f32, tag=f"tfu{i}", name=f"t_full{i}",
                            addr_space="Shared") for i in range(2)]
        groups = [list(range(NC))]

        # ---- phase 1: layer-1 table for own shard, from xT input
        for w in range(NW):
            rows = min(P, SH - w * P)
            xt = sb.tile([cfg.in_dim, P], f32, tag="xt")
            nc.sync.dma_start(out=xt[:, :rows], in_=xT[:, w * P:w * P + rows])
            ps = psum.tile([P, TS], f32, tag="tps")
            nc.tensor.matmul(out=ps[:rows, :], lhsT=xt[:, :rows], rhs=w1e_sb[:],
                             start=True, stop=True)
            tsb = sb.tile([P, TS], f32, tag="tsb")
            nc.scalar.copy(out=tsb[:rows, :], in_=ps[:rows, :])
            nc.sync.dma_start(out=t_shard[0][w * P:w * P + rows, :],
                              in_=tsb[:rows, :])

        nc.gpsimd.collective_compute(
            "AllGather", mybir.AluOpType.bypass, replica_groups=groups,
            ins=[t_shard[0][:, :]], outs=[t_full[0][:, :]])

        # ---- edge phase (shared between the two layers)
        def edge_phase(layer):
            import os
            nwin_lim = int(os.environ.get("GAT_NWIN", "1000000"))
            nogather = "nogather" in os.environ.get("GAT_PARTS", "")
            tf, tsh = t_full[layer], t_shard[layer]
            gcol = 0
            ccol = 0
            for w in range(NW):
                if w >= nwin_lim:
                    break
                rows = min(P, SH - w * P)
                nch = NCH[w]
                G = eps.tile([P, nch * TS], f32, tag="G")
                ALd = eps.tile([P, nch * 64], f32, tag="ALd")
                ST = eps.tile([P, nch * 128], f32, tag="ST")
                score = eps.tile([P, nch * 4], f32, tag="score")

                # gathers: src rows (lo/hi) from full table, al rows local
                parts = [(plan["nch_lo"][w], 0), (plan["nch_hi"][w], LO)]
                coff = 0
                gc = gcol
                MAXCK = int(os.environ.get("GAT_MAXCK", "4"))
                if nogather:
                    nc.vector.memset(G[:], 0.001)
                    nc.vector.memset(ALd[:], 0.001)
                    gc += 8 * nch
                else:
                    for nck, base in parts:
                        nrows = min(N, LO) if base == 0 else HI_ROWS
                        for c0 in range(0, nck, MAXCK):
                            cn = min(MAXCK, nck - c0)
                            nc.gpsimd.dma_gather(
                                out_ap=_ap(G[:], (coff + c0) * TS,
                                           [[TS, cn], [1, TS]]),
                                in_ap=_apd(tf[:], base * TS,
                                           [[TS, nrows], [1, TS]]),
                                idxs_ap=gidx_sb[:, gc + 8 * c0:gc + 8 * (c0 + cn)],
                                num_idxs=cn * P, num_idxs_reg=cn * P,
                                elem_size=TS, elem_step=TS)
                        coff += nck
                        gc += 8 * nck
                    for c0 in range(0, nch, MAXCK):
                        cn = min(MAXCK, nch - c0)
                        nc.gpsimd.dma_gather(
                            out_ap=_ap(ALd[:], c0 * 64, [[64, cn], [1, 64]]),
                            in_ap=_apd(tsh[:], D1, [[TS, SH], [1, 64]]),
                            idxs_ap=alidx_sb[:, gcol + 8 * c0:gcol + 8 * (c0 + cn)],
                            num_idxs=cn * P, num_idxs_reg=cn * P,
                            elem_size=64, elem_step=TS)

                # one-hot mask ST[e, (chunk), d] = (dstoff == d)
                nc.vector.tensor_tensor(
                    out=_ap(ST[:], 0, [[128, nch], [1, 128]]),
                    in0=_ap(dstoff_sb[:], ccol, [[1, nch], [0, 128]]),
                    in1=_ap(iotaf_sb[:], 0, [[0, nch], [1, 128]]),
                    op=mybir.AluOpType.is_equal)

                # scores: ex = exp(leaky_relu(al_src + al_dst))
                nc.vector.tensor_tensor(
                    out=_ap(score[:], 0, [[4, nch], [1, 4]]),
                    in0=_ap(G[:], D1, [[TS, nch], [1, 4]]),
                    in1=_ap(ALd[:], 4, [[64, nch], [1, 4]]),
                    op=mybir.AluOpType.add)
                nc.vector.scalar_tensor_tensor(
                    out=_ap(score[:], 0, [[4, nch], [1, 4]]),
                    in0=_ap(score[:], 0, [[4, nch], [1, 4]]),
                    scalar=0.2,
                    in1=_ap(score[:], 0, [[4, nch], [1, 4]]),
                    op0=mybir.AluOpType.mult, op1=mybir.AluOpType.max)
                nc.scalar.activation(
                    out=_ap(G[:], D1, [[TS, nch], [1, 4]]),
                    in_=_ap(score[:], 0, [[4, nch], [1, 4]]),
                    func=mybir.ActivationFunctionType.Exp)

                # weight gathered h rows by ex (per head), in place
                for h in range(H):
                    nc.vector.tensor_tensor(
                        out=_ap(G[:], h * C, [[TS, nch], [1, C]]),
                        in0=_ap(G[:], h * C, [[TS, nch], [1, C]]),
                        in1=_ap(G[:], D1 + h, [[TS, nch], [0, C]]),
                        op=mybir.AluOpType.mult)

                # segment sum: psum[d, 0:260] += ST_c^T @ G_c
                agg = psum.tile([P, D1 + 4], f32, tag="agg")
                for cchunk in range(nch):
                    nc.tensor.matmul(
                        out=agg[:, :],
                        lhsT=ST[:, cchunk * 128:(cchunk + 1) * 128],
                        rhs=G[:, cchunk * TS:cchunk * TS + D1 + 4],
                        start=(cchunk == 0), stop=(cchunk == nch - 1))

                # normalize + relu (+ next-layer table / output write)
                den = eps.tile([P, 4], f32, tag="den")
                nc.vector.tensor_scalar_max(out=den[:], in0=agg[:, D1:D1 + 4],
                                            scalar1=1e-30)
                rec = eps.tile([P, 4], f32, tag="rec")
                nc.vector.reciprocal(out=rec[:], in_=den[:])
                act = eps.tile([P, D1], f32, tag="act")
                for h in range(H):
                    nc.scalar.activation(
                        out=act[:rows, h * C:(h + 1) * C],
                        in_=agg[:rows, h * C:(h + 1) * C],
                        func=mybir.ActivationFunctionType.Relu,
                        scale=rec[:rows, h:h + 1])

                if layer == 0:
                    # layer-2 table rows: transpose act, matmul with w2e
                    tp = psum.tile([P, D1], f32, tag="tp")
                    xT2 = eps.tile([P, D1], f32, tag="xT2")
                    for k in range(k2_tiles):
                        nc.tensor.transpose(
                            out=tp[:, k * P:k * P + rows],
                            in_=act[:rows, k * P:(k + 1) * P],
                            identity=ident[:rows, :rows])
                    for k in range(k2_tiles):
                        nc.vector.tensor_copy(
                            out=xT2[:, k * P:k * P + rows],
                            in_=tp[:, k * P:k * P + rows])
                    t2p = psum.tile([P, TS], f32, tag="t2p")
                    for k in range(k2_tiles):
                        nc.tensor.matmul(
                            out=t2p[:rows, :],
                            lhsT=xT2[:, k * P:k * P + rows],
                            rhs=w2e_sb[k][:],
                            start=(k == 0), stop=(k == k2_tiles - 1))
                    t2sb = eps.tile([P, TS], f32, tag="t2sb")
                    nc.scalar.copy(out=t2sb[:rows, :], in_=t2p[:rows, :])
                    nc.sync.dma_start(out=t_shard[1][w * P:w * P + rows, :],
                                      in_=t2sb[:rows, :])
                else:
                    nc.sync.dma_start(out=out_d[w * P:w * P + rows, :],
                                      in_=act[:rows, :])

                gcol = gc
                ccol += nch

        import os
        _skip = os.environ.get("GAT_SKIP", "")
        if "e0" not in _skip:
            edge_phase(0)
        if "ag2" not in _skip:
            nc.gpsimd.collective_compute(
                "AllGather", mybir.AluOpType.bypass, replica_groups=groups,
                ins=[t_shard[1][:, :]], outs=[t_full[1][:, :]])
        if "e1" not in _skip:
            edge_phase(1)

    nc.compile()  # Bacc legalization: wait relocation, library loads, ISA bytes
    return nc


def _make_inputs(cfg, plan, per_core, x, W1, a1s, a1d, W2, a2s, a2d):
    iotaf = np.tile(np.arange(P, dtype=np.float32), (P, 1))
    w1e = _pack_wext(cfg, np.asarray(W1, np.float32), np.asarray(a1s, np.float32),
                     np.asarray(a1d, np.float32))
    w2e = _pack_wext(cfg, np.asarray(W2, np.float32), np.asarray(a2s, np.float32),
                     np.asarray(a2d, np.float32))
    x = np.asarray(x, np.float32)
    in_maps = []
    for c in range(cfg.n_cores):
        xs = x[c * cfg.shard:(c + 1) * cfg.shard].T.copy()
        in_maps.append(dict(
            xT=xs, w1e=w1e, w2e=w2e, iotaf=iotaf,
            gidx=per_core[c]["gidx"], alidx=per_core[c]["alidx"],
            dstoff=per_core[c]["dstoff"]))
    return in_maps


def _ensure_ntff_hook():
    """Register the axon NTFF profiling hook if the antenv shim is absent."""
    import types
    try:
        from antenv.axon_hooks import get_axon_ntff_profile_hook  # noqa: F401
        return
    except ImportError:
        pass
    import antenv
    mod = types.ModuleType("antenv.axon_hooks")
    _h = [None]
    mod.set_axon_ntff_profile_hook = lambda h: _h.__setitem__(0, h)
    mod.get_axon_ntff_profile_hook = lambda: _h[0]
    sys.modules["antenv.axon_hooks"] = mod
    antenv.axon_hooks = mod
    try:
        from trn_agent_boot.trn_boot import _ntff_profile_via_ctypes
        mod.set_axon_ntff_profile_hook(
            _ntff_profile_via_ctypes("/opt/axon/libaxon_pjrt.so"))
    except Exception:
        pass


def run(cfg, inputs, trace=False):
    from concourse.bass_utils import run_bass_kernel_spmd

    if trace:
        _ensure_ntff_hook()

    plan, per_core = _plan_edges(cfg, np.asarray(inputs["edge_index"]))
    nc = build_program(cfg, plan)
    in_maps = _make_inputs(cfg, plan, per_core, inputs["x"],
                           inputs["W1"], inputs["a1_src"], inputs["a1_dst"],
                           inputs["W2"], inputs["a2_src"], inputs["a2_dst"])
    b1 = np.asarray(inputs["b1"], np.float32)
    b2 = np.asarray(inputs["b2"], np.float32)
    assert not (np.any(b1) or np.any(b2)), "nonzero biases not supported"
    res = run_bass_kernel_spmd(nc, in_maps, list(range(cfg.n_cores)),
                               trace=trace)
    out = np.concatenate([res.results[c]["out"] for c in range(cfg.n_cores)],
                         axis=0)
    return out, res


def kernel(**inputs) -> np.ndarray:
    cfg = Cfg()
    assert inputs["x"].shape == (cfg.n_nodes, cfg.in_dim)
    out, _ = run(cfg, inputs, trace=False)
    return out.astype(np.float32)



# revision 16
# speedup vs baseline: 1.1974x; 1.1974x over previous
"""GAT 2-layer encoder kernel for Trainium2 (8 NeuronCores, Bass/Tile).

Strategy (graph/data parallel, dst-sharded), v2:
  - Nodes sharded contiguously across 8 cores (6250 each); each core owns the
    edges whose destination lands in its shard (plus self loops).
  - Per layer, each core computes an fp16 "node table" for its shard:
        row n = [ h(n) (256 f16) | al_src(n) (4) | al_dst(n) (4) | pad to 384 ]
    Tables are AllGather'd so every core has the full [N, 384] f16 table.
  - Edge phase runs on groups of SW destination windows (128 dst each).
    Per group: three ASYNC dma_gather preps (prepare_only=True) — source table
    rows from the lo/hi halves of the full table (int16 idx limit) and the
    al_dst rows from the local shard table — fired by one trigger_dma.  The
    GpSimd engine only pays descriptor generation; transfers overlap compute.
  - Scores ex = exp(leaky_relu(al_src + al_dst)) computed in fp16 on
    vector/scalar; gathered h rows scaled by ex in one 3-dim-AP vector op;
    per-window segment-sum via PE matmuls (fp16 in, fp32 PSUM accumulate):
    onehot(dst)^T @ [ex*h | ex].  Normalization fused into Relu activations.
  - Layer-1 epilogue transposes activations and computes layer-2 table rows;
    layer-2 epilogue writes final f32 output rows.

Edge structure (indices, chunk counts) is baked in at build time; per-window
chunk counts are maxed across cores so one SPMD program runs on all 8 cores
with per-core index data.
"""

import math
import sys

import numpy as np

sys.path.insert(0, "/opt/trn_rl_repo")

P = 128  # partitions


class Cfg:
    def __init__(self, n_nodes=50000, in_dim=128, heads=4, hid=64,
                 n_cores=8, lo_split=32768, sw=2):
        self.n_nodes = n_nodes
        self.in_dim = in_dim
        self.heads = heads
        self.hid = hid
        self.n_cores = n_cores
        self.d1 = heads * hid                       # 256
        self.ts = 384                               # f16 table row stride
        self.lo_split = lo_split                    # int16-safe table split
        self.sw = sw                                # windows per gather group
        assert n_nodes % n_cores == 0
        self.shard = n_nodes // n_cores             # 6250
        self.nw = math.ceil(self.shard / P)         # windows per core (49)
        self.shard_pad = self.nw * P


def _plan_edges(cfg, edge_index):
    """Host-side: per-core, per-group padded edge lists in gather layout.

    Chunk slot order within a group: all lo chunks (window-major), then all
    hi chunks (window-major).  Per-window chunk counts are maxed across cores
    so the slot structure is SPMD-uniform; indices/offsets are per-core data.

    Returns (plan, per_core):
      plan.groups: list of dicts with nlo, nhi, nch, win (list of per-window
        dicts: w, rows, slots)
      per_core: dicts with gidx/alidx [128, NCOLS] int16, dstoff [128, NCHTOT]
        f16
    """
    NC, SH, NW, SW = cfg.n_cores, cfg.shard, cfg.nw, cfg.sw
    src = np.asarray(edge_index[0], dtype=np.int64)
    dst = np.asarray(edge_index[1], dtype=np.int64)
    loops = np.arange(cfg.n_nodes, dtype=np.int64)
    src = np.concatenate([src, loops])
    dst = np.concatenate([dst, loops])

    core = dst // SH
    win = (dst - core * SH) // P

    order = np.lexsort((src, win, core))
    src_s, dst_s, core_s, win_s = src[order], dst[order], core[order], win[order]
    key = core_s * NW + win_s
    starts = np.searchsorted(key, np.arange(NC * NW))
    ends = np.searchsorted(key, np.arange(NC * NW) + 1)

    lo_edges = [[None] * NW for _ in range(NC)]
    hi_edges = [[None] * NW for _ in range(NC)]
    for c in range(NC):
        for w in range(NW):
            s, e = starts[c * NW + w], ends[c * NW + w]
            es, ed = src_s[s:e], dst_s[s:e]
            lo = es < cfg.lo_split
            lo_edges[c][w] = (es[lo], ed[lo])
            hi_edges[c][w] = (es[~lo], ed[~lo])

    nch_lo = [0] * NW
    nch_hi = [0] * NW
    for w in range(NW):
        ml = max(len(lo_edges[c][w][0]) for c in range(NC))
        mh = max(len(hi_edges[c][w][0]) for c in range(NC))
        nch_lo[w] = math.ceil(ml / P) if ml else 0
        nch_hi[w] = math.ceil(mh / P) if mh else 0
        if nch_lo[w] == 0 and nch_hi[w] == 0:
            nch_lo[w] = 1  # degenerate empty window: keep shapes legal

    # group windows into superwindows of SW
    groups = []
    for w0 in range(0, NW, SW):
        ws = list(range(w0, min(w0 + SW, NW)))
        nlo = sum(nch_lo[w] for w in ws)
        nhi = sum(nch_hi[w] for w in ws)
        winfo = []
        lo_off = 0
        hi_off = nlo
        for w in ws:
            slots = (list(range(lo_off, lo_off + nch_lo[w]))
                     + list(range(hi_off, hi_off + nch_hi[w])))
            lo_off += nch_lo[w]
            hi_off += nch_hi[w]
            winfo.append(dict(w=w, rows=min(P, SH - w * P), slots=slots))
        groups.append(dict(nlo=nlo, nhi=nhi, nch=nlo + nhi, win=winfo))

    nch_tot = sum(g["nch"] for g in groups)
    ncols = 8 * nch_tot  # idx cols per core: (nch*128)/16

    def wrap16(vals, n_idx):
        """[n_idx] int -> [128, n_idx//16] int16 in dma_gather layout."""
        cols = n_idx // 16
        out = np.zeros((16, cols), dtype=np.int16)
        v = np.asarray(vals, dtype=np.int64)
        out[np.arange(n_idx) % 16, np.arange(n_idx) // 16] = v
        return np.tile(out, (8, 1))

    per_core = []
    for c in range(NC):
        gidx = np.zeros((P, ncols), dtype=np.int16)
        alidx = np.zeros((P, ncols), dtype=np.int16)
        dstoff = np.full((P, nch_tot), 255.0, dtype=np.float16)
        gcol = 0
        ccol = 0
        for g in groups:
            ws = [wi["w"] for wi in g["win"]]
            # slot-ordered per-chunk data: lo chunks (window-major) then hi
            gvals = []   # table row index (region-relative)
            avals = []   # local dst row index
            ovals = []   # dst offset within window (or 255 pad)
            for region, base in ((lo_edges, 0), (hi_edges, cfg.lo_split)):
                counts = nch_lo if base == 0 else nch_hi
                for w in ws:
                    nchunks = counts[w]
                    if nchunks == 0:
                        continue
                    es, ed = region[c][w]
                    n_idx = nchunks * P
                    gv = np.zeros(n_idx, dtype=np.int64)
                    av = np.zeros(n_idx, dtype=np.int64)
                    ov = np.full(n_idx, 255.0, dtype=np.float16)
                    k = len(es)
                    gv[:k] = es - base
                    d_local = ed - c * SH
                    av[:k] = d_local
                    ov[:k] = (d_local - w * P).astype(np.float16)
                    gvals.append(gv)
                    avals.append(av)
                    ovals.append(ov)
            gv = np.concatenate(gvals)
            av = np.concatenate(avals)
            ov = np.concatenate(ovals)
            n_idx = len(gv)
            assert n_idx == g["nch"] * P
            gidx[:, gcol:gcol + n_idx // 16] = wrap16(gv, n_idx)
            alidx[:, gcol:gcol + n_idx // 16] = wrap16(av, n_idx)
            dstoff[:, ccol:ccol + g["nch"]] = ov.reshape(g["nch"], P).T
            gcol += n_idx // 16
            ccol += g["nch"]
        assert gcol == ncols and ccol == nch_tot
        per_core.append(dict(gidx=gidx, alidx=alidx, dstoff=dstoff))

    plan = dict(groups=groups, nch_tot=nch_tot, ncols=ncols)
    return plan, per_core


def _pack_wext(cfg, W, a_src, a_dst):
    """[K, 256] weight -> [K, 384] f16: [W | W@Asrc | W@Adst | 0]."""
    K = W.shape[0]
    H, C = cfg.heads, cfg.hid
    out = np.zeros((K, cfg.ts), dtype=np.float32)
    out[:, :cfg.d1] = W
    for h in range(H):
        out[:, cfg.d1 + h] = W[:, h * C:(h + 1) * C] @ a_src[h]
        out[:, cfg.d1 + 4 + h] = W[:, h * C:(h + 1) * C] @ a_dst[h]
    return out.astype(np.float16)


def _ap(t, offset_elems, free_pattern):
    """SBUF AP with explicit free [step, count] dims on top of a tile AP."""
    import concourse.bass as bass
    return bass.AP(t.tensor, t.offset + offset_elems,
                   [list(t.ap[0])] + [list(p) for p in free_pattern])


def _apd(t, offset_elems, pattern):
    """DRAM AP with fully explicit [step, count] dims (no partition dim)."""
    import concourse.bass as bass
    return bass.AP(t.tensor, t.offset + offset_elems,
                   [list(p) for p in pattern])


def build_program(cfg, plan):
    import concourse.bass as bass
    import concourse.mybir as mybir
    import concourse.tile as tile
    from concourse import bacc
    from concourse.masks import make_identity
    from contextlib import ExitStack

    f32 = mybir.dt.float32
    f16 = mybir.dt.float16
    i16 = mybir.dt.int16
    TS, D1, H, C = cfg.ts, cfg.d1, cfg.heads, cfg.hid
    SH, NW, NC = cfg.shard, cfg.nw, cfg.n_cores
    GROUPS = plan["groups"]
    NCOLS = plan["ncols"]
    LO = cfg.lo_split
    N = cfg.n_nodes
    HI_ROWS = N - LO
    k2_tiles = D1 // P            # 2 for layer 2

    nc = bacc.Bacc(dynamic_dma_scratch_size=65536)

    xT = nc.dram_tensor("xT", [cfg.in_dim, SH], f16, kind="ExternalInput")
    w1e = nc.dram_tensor("w1e", [cfg.in_dim, TS], f16, kind="ExternalInput")
    w2e = nc.dram_tensor("w2e", [D1, TS], f16, kind="ExternalInput")
    gidx_d = nc.dram_tensor("gidx", [P, NCOLS], i16, kind="ExternalInput")
    alidx_d = nc.dram_tensor("alidx", [P, NCOLS], i16, kind="ExternalInput")
    dstoff_d = nc.dram_tensor("dstoff", [P, plan["nch_tot"]], f16,
                              kind="ExternalInput")
    iotaf_d = nc.dram_tensor("iotaf", [P, P], f16, kind="ExternalInput")
    out_d = nc.dram_tensor("out", [SH, D1], f32, kind="ExternalOutput")

    with ExitStack() as ctx:
        tc = ctx.enter_context(tile.TileContext(nc))
        const = ctx.enter_context(tc.tile_pool(name="const", bufs=1))
        sb = ctx.enter_context(tc.tile_pool(name="sb", bufs=2))
        eps = ctx.enter_context(tc.tile_pool(name="eps", bufs=2))
        psum = ctx.enter_context(tc.tile_pool(name="psum", bufs=2, space="PSUM"))
        dram = ctx.enter_context(tc.tile_pool(name="dram", bufs=1, space="DRAM"))

        # ---- constants / static inputs into SBUF
        w1e_sb = const.tile([cfg.in_dim, TS], f16)
        nc.sync.dma_start(out=w1e_sb[:], in_=w1e[:, :])
        w2e_sb = [const.tile([P, TS], f16, tag=f"w2e{k}", name=f"w2e_sb{k}")
                  for k in range(k2_tiles)]
        for k in range(k2_tiles):
            nc.sync.dma_start(out=w2e_sb[k][:], in_=w2e[k * P:(k + 1) * P, :])
        gidx_sb = const.tile([P, NCOLS], i16)
        nc.sync.dma_start(out=gidx_sb[:], in_=gidx_d[:, :])
        alidx_sb = const.tile([P, NCOLS], i16)
        nc.sync.dma_start(out=alidx_sb[:], in_=alidx_d[:, :])
        dstoff_sb = const.tile([P, plan["nch_tot"]], f16)
        nc.sync.dma_start(out=dstoff_sb[:], in_=dstoff_d[:, :])
        iotaf_sb = const.tile([P, P], f16)
        nc.sync.dma_start(out=iotaf_sb[:], in_=iotaf_d[:, :])
        ident = const.tile([P, P], f16)
        make_identity(nc, ident[:])


        t_shard = [dram.tile([SH, TS], f16, tag=f"tsh{i}", name=f"t_shard{i}")
                   for i in range(2)]
        t_full = [dram.tile([N, TS], f16, tag=f"tfu{i}", name=f"t_full{i}",
                            addr_space="Shared") for i in range(2)]
        cgroups = [list(range(NC))]

        # ---- phase 1: layer-1 table for own shard, from xT input
        for w in range(NW):
            rows = min(P, SH - w * P)
            xt = sb.tile([cfg.in_dim, P], f16, tag="xt")
            nc.sync.dma_start(out=xt[:, :rows], in_=xT[:, w * P:w * P + rows])
            ps = psum.tile([P, TS], f32, tag="tps")
            nc.tensor.matmul(out=ps[:rows, :], lhsT=xt[:, :rows], rhs=w1e_sb[:],
                             start=True, stop=True)
            tsb = sb.tile([P, TS], f16, tag="tsb")
            nc.scalar.copy(out=tsb[:rows, :], in_=ps[:rows, :])
            nc.sync.dma_start(out=t_shard[0][w * P:w * P + rows, :],
                              in_=tsb[:rows, :])

        nc.gpsimd.collective_compute(
            "AllGather", mybir.AluOpType.bypass, replica_groups=cgroups,
            ins=[t_shard[0][:, :]], outs=[t_full[0][:, :]])

        # ---- edge phase (shared between the two layers)

        def edge_phase(layer):
            import os
            ngrp = int(os.environ.get("GAT_NGRP", "1000000"))
            parts = os.environ.get("GAT_PARTS", "")
            tf, tsh = t_full[layer], t_shard[layer]
            gcol = 0
            ccol = 0
            for gn, g in enumerate(GROUPS):
                if gn >= ngrp:
                    break
                nch, nlo, nhi = g["nch"], g["nlo"], g["nhi"]
                G = eps.tile([P, nch * TS], f16, tag="G")
                ALd = eps.tile([P, nch * P], f16, tag="ALd")
                ST = eps.tile([P, nch * P], f16, tag="ST")

                if "nogather" in parts:
                    nc.vector.memset(G[:], 0.001)
                    nc.vector.memset(ALd[:], 0.001)
                # gathers: src rows (lo/hi) from full table, al rows
                # local.  Real ucode burns ~1 ring descriptor per index
                # (carveout = dynamic_dma_scratch_size/16), so cap each call
                # at MAXCK chunks.
                MAXCK = 8
                for nck, coff, base in (() if "nogather" in parts else
                                        ((nlo, 0, 0), (nhi, nlo, LO))):
                    if nck == 0:
                        continue
                    nrows = min(N, LO) if base == 0 else HI_ROWS
                    for c0 in range(0, nck, MAXCK):
                        cn = min(MAXCK, nck - c0)
                        gc = gcol + 8 * (coff + c0)
                        nc.gpsimd.dma_gather(
                            out_ap=_ap(G[:], (coff + c0) * TS,
                                       [[TS, cn], [1, TS]]),
                            in_ap=_apd(tf[:], base * TS,
                                       [[TS, nrows], [1, TS]]),
                            idxs_ap=gidx_sb[:, gc:gc + 8 * cn],
                            num_idxs=cn * P, num_idxs_reg=cn * P,
                            elem_size=TS, elem_step=TS)
                if "nogather" not in parts:
                  for c0 in range(0, nch, MAXCK):
                    cn = min(MAXCK, nch - c0)
                    nc.gpsimd.dma_gather(
                        out_ap=_ap(ALd[:], c0 * P, [[P, cn], [1, P]]),
                        in_ap=_apd(tsh[:], D1, [[TS, SH], [1, P]]),
                        idxs_ap=alidx_sb[:, gcol + 8 * c0:gcol + 8 * (c0 + cn)],
                        num_idxs=cn * P, num_idxs_reg=cn * P,
                        elem_size=P, elem_step=TS)

                # one-hot mask ST[e, (chunk), d] = (dstoff == d)
                nc.vector.tensor_tensor(
                    out=_ap(ST[:], 0, [[P, nch], [1, P]]),
                    in0=_ap(dstoff_sb[:], ccol, [[1, nch], [0, P]]),
                    in1=_ap(iotaf_sb[:], 0, [[0, nch], [1, P]]),
                    op=mybir.AluOpType.is_equal)

                # scores in G[:, D1:D1+4]: ex = exp(leaky_relu(al_src+al_dst))
                if "noscore" not in parts:
                  nc.vector.tensor_tensor(
                    out=_ap(G[:], D1, [[TS, nch], [1, 4]]),
                    in0=_ap(G[:], D1, [[TS, nch], [1, 4]]),
                    in1=_ap(ALd[:], 4, [[P, nch], [1, 4]]),
                    op=mybir.AluOpType.add)
                  nc.vector.scalar_tensor_tensor(
                    out=_ap(G[:], D1, [[TS, nch], [1, 4]]),
                    in0=_ap(G[:], D1, [[TS, nch], [1, 4]]),
                    scalar=0.2,
                    in1=_ap(G[:], D1, [[TS, nch], [1, 4]]),
                    op0=mybir.AluOpType.mult, op1=mybir.AluOpType.max)
                  nc.scalar.activation(
                    out=_ap(G[:], D1, [[TS, nch], [1, 4]]),
                    in_=_ap(G[:], D1, [[TS, nch], [1, 4]]),
                    func=mybir.ActivationFunctionType.Exp)

                # weight gathered h rows by ex (all heads, one 3-dim-AP op)
                if "noex" not in parts:
                  nc.vector.tensor_tensor(
                    out=_ap(G[:], 0, [[TS, nch], [C, H], [1, C]]),
                    in0=_ap(G[:], 0, [[TS, nch], [C, H], [1, C]]),
                    in1=_ap(G[:], D1, [[TS, nch], [1, H], [0, C]]),
                    op=mybir.AluOpType.mult)

                for wi in g["win"]:
                    w, rows, slots = wi["w"], wi["rows"], wi["slots"]
                    # segment sum: psum[d, 0:260] += ST_c^T @ G_c
                    agg = psum.tile([P, D1 + 4], f32, tag="agg")
                    if "nomm" in parts:
                        nc.vector.memset(agg[:], 0.5)
                    for i, slot in enumerate(() if "nomm" in parts else slots):
                        nc.tensor.matmul(
                            out=agg[:, :],
                            lhsT=ST[:, slot * P:(slot + 1) * P],
                            rhs=G[:, slot * TS:slot * TS + D1 + 4],
                            start=(i == 0), stop=(i == len(slots) - 1))

                    # normalize + relu (+ next-layer table / output write)
                    den = eps.tile([P, 4], f32, tag="den")
                    nc.vector.tensor_scalar_max(out=den[:], in0=agg[:, D1:D1 + 4],
                                                scalar1=1e-30)
                    rec = eps.tile([P, 4], f32, tag="rec")
                    nc.vector.reciprocal(out=rec[:], in_=den[:])

                    if layer == 0:
                        act = eps.tile([P, D1], f16, tag="act")
                        for h in range(H):
                            nc.scalar.activation(
                                out=act[:rows, h * C:(h + 1) * C],
                                in_=agg[:rows, h * C:(h + 1) * C],
                                func=mybir.ActivationFunctionType.Relu,
                                scale=rec[:rows, h:h + 1])
                        # layer-2 table rows: transpose act, matmul with w2e
                        tp = psum.tile([P, D1], f16, tag="tp")
                        xT2 = eps.tile([P, D1], f16, tag="xT2")
                        if "notp" in parts:
                            nc.vector.memset(xT2[:], 0.1)
                        for k in range(() and 0 or (0 if "notp" not in parts else k2_tiles), k2_tiles):
                            nc.tensor.transpose(
                                out=tp[:, k * P:k * P + rows],
                                in_=act[:rows, k * P:(k + 1) * P],
                                identity=ident[:rows, :rows])
                        for k in range(k2_tiles if "notp" in parts else 0, k2_tiles):
                            nc.vector.tensor_copy(
                                out=xT2[:, k * P:k * P + rows],
                                in_=tp[:, k * P:k * P + rows])
                        t2p = psum.tile([P, TS], f32, tag="t2p")
                        for k in range(k2_tiles):
                            nc.tensor.matmul(
                                out=t2p[:rows, :],
                                lhsT=xT2[:, k * P:k * P + rows],
                                rhs=w2e_sb[k][:],
                                start=(k == 0), stop=(k == k2_tiles - 1))
                        t2sb = eps.tile([P, TS], f16, tag="t2sb")
                        nc.scalar.copy(out=t2sb[:rows, :], in_=t2p[:rows, :])
                        nc.sync.dma_start(out=t_shard[1][w * P:w * P + rows, :],
                                          in_=t2sb[:rows, :])
                    else:
                        act = eps.tile([P, D1], f32, tag="act")
                        for h in range(H):
                            nc.scalar.activation(
                                out=act[:rows, h * C:(h + 1) * C],
                                in_=agg[:rows, h * C:(h + 1) * C],
                                func=mybir.ActivationFunctionType.Relu,
                                scale=rec[:rows, h:h + 1])
                        nc.sync.dma_start(out=out_d[w * P:w * P + rows, :],
                                          in_=act[:rows, :])

                gcol += 8 * nch
                ccol += nch

        import os
        _skip = os.environ.get("GAT_SKIP", "")
        if "e0" not in _skip:
            edge_phase(0)
        if "ag2" not in _skip:
            nc.gpsimd.collective_compute(
                "AllGather", mybir.AluOpType.bypass, replica_groups=cgroups,
                ins=[t_shard[1][:, :]], outs=[t_full[1][:, :]])
        if "e1" not in _skip:
            edge_phase(1)

    nc.compile()  # Bacc legalization: wait relocation, library loads, ISA bytes
    return nc


def _make_inputs(cfg, plan, per_core, x, W1, a1s, a1d, W2, a2s, a2d):
    iotaf = np.tile(np.arange(P, dtype=np.float16), (P, 1))
    w1e = _pack_wext(cfg, np.asarray(W1, np.float32), np.asarray(a1s, np.float32),
                     np.asarray(a1d, np.float32))
    w2e = _pack_wext(cfg, np.asarray(W2, np.float32), np.asarray(a2s, np.float32),
                     np.asarray(a2d, np.float32))
    x = np.asarray(x, np.float32)
    in_maps = []
    for c in range(cfg.n_cores):
        xs = x[c * cfg.shard:(c + 1) * cfg.shard].T.astype(np.float16).copy()
        in_maps.append(dict(
            xT=xs, w1e=w1e, w2e=w2e, iotaf=iotaf,
            gidx=per_core[c]["gidx"], alidx=per_core[c]["alidx"],
            dstoff=per_core[c]["dstoff"]))
    return in_maps


def _ensure_ntff_hook():
    """Register the axon NTFF profiling hook if the antenv shim is absent."""
    import types
    try:
        from antenv.axon_hooks import get_axon_ntff_profile_hook  # noqa: F401
        return
    except ImportError:
        pass
    import antenv
    mod = types.ModuleType("antenv.axon_hooks")
    _h = [None]
    mod.set_axon_ntff_profile_hook = lambda h: _h.__setitem__(0, h)
    mod.get_axon_ntff_profile_hook = lambda: _h[0]
    sys.modules["antenv.axon_hooks"] = mod
    antenv.axon_hooks = mod
    try:
        from trn_agent_boot.trn_boot import _ntff_profile_via_ctypes
        mod.set_axon_ntff_profile_hook(
            _ntff_profile_via_ctypes("/opt/axon/libaxon_pjrt.so"))
    except Exception:
        pass


def run(cfg, inputs, trace=False):
    from concourse.bass_utils import run_bass_kernel_spmd

    if trace:
        _ensure_ntff_hook()

    plan, per_core = _plan_edges(cfg, np.asarray(inputs["edge_index"]))
    nc = build_program(cfg, plan)
    in_maps = _make_inputs(cfg, plan, per_core, inputs["x"],
                           inputs["W1"], inputs["a1_src"], inputs["a1_dst"],
                           inputs["W2"], inputs["a2_src"], inputs["a2_dst"])
    b1 = np.asarray(inputs["b1"], np.float32)
    b2 = np.asarray(inputs["b2"], np.float32)
    assert not (np.any(b1) or np.any(b2)), "nonzero biases not supported"
    res = run_bass_kernel_spmd(nc, in_maps, list(range(cfg.n_cores)),
                               trace=trace)
    out = np.concatenate([res.results[c]["out"] for c in range(cfg.n_cores)],
                         axis=0)
    return out, res


def kernel(**inputs) -> np.ndarray:
    cfg = Cfg()
    assert inputs["x"].shape == (cfg.n_nodes, cfg.in_dim)
    out, _ = run(cfg, inputs, trace=False)
    return out.astype(np.float32)


# revision 19
# speedup vs baseline: 2.3650x; 1.9751x over previous
"""GAT 2-layer encoder kernel for Trainium2 (8 NeuronCores, Bass/Tile).

Strategy (graph/data parallel, dst-sharded), v2:
  - Nodes sharded contiguously across 8 cores (6250 each); each core owns the
    edges whose destination lands in its shard (plus self loops).
  - Per layer, each core computes an fp16 "node table" for its shard:
        row n = [ h(n) (256 f16) | al_src(n) (4) | al_dst(n) (4) | pad to 384 ]
    Tables are AllGather'd so every core has the full [N, 384] f16 table.
  - Edge phase runs on groups of SW destination windows (128 dst each).
    Per group: three ASYNC dma_gather preps (prepare_only=True) — source table
    rows from the lo/hi halves of the full table (int16 idx limit) and the
    al_dst rows from the local shard table — fired by one trigger_dma.  The
    GpSimd engine only pays descriptor generation; transfers overlap compute.
  - Scores ex = exp(leaky_relu(al_src + al_dst)) computed in fp16 on
    vector/scalar; gathered h rows scaled by ex in one 3-dim-AP vector op;
    per-window segment-sum via PE matmuls (fp16 in, fp32 PSUM accumulate):
    onehot(dst)^T @ [ex*h | ex].  Normalization fused into Relu activations.
  - Layer-1 epilogue transposes activations and computes layer-2 table rows;
    layer-2 epilogue writes final f32 output rows.

Edge structure (indices, chunk counts) is baked in at build time; per-window
chunk counts are maxed across cores so one SPMD program runs on all 8 cores
with per-core index data.
"""

import math
import sys

import numpy as np

sys.path.insert(0, "/opt/trn_rl_repo")

P = 128  # partitions


class Cfg:
    def __init__(self, n_nodes=50000, in_dim=128, heads=4, hid=64,
                 n_cores=8, lo_split=32768, sw=2):
        self.n_nodes = n_nodes
        self.in_dim = in_dim
        self.heads = heads
        self.hid = hid
        self.n_cores = n_cores
        self.d1 = heads * hid                       # 256
        self.ts = 384                               # f16 table row stride
        self.lo_split = lo_split                    # int16-safe table split
        self.sw = sw                                # windows per gather group
        assert n_nodes % n_cores == 0
        self.shard = n_nodes // n_cores             # 6250
        self.nw = math.ceil(self.shard / P)         # windows per core (49)
        self.shard_pad = self.nw * P


def _plan_edges(cfg, edge_index):
    """Host-side: per-core, per-group padded edge lists in gather layout.

    Chunk slot order within a group: all lo chunks (window-major), then all
    hi chunks (window-major).  Per-window chunk counts are maxed across cores
    so the slot structure is SPMD-uniform; indices/offsets are per-core data.

    Returns (plan, per_core):
      plan.groups: list of dicts with nlo, nhi, nch, win (list of per-window
        dicts: w, rows, slots)
      per_core: dicts with gidx/alidx [128, NCOLS] int16, dstoff [128, NCHTOT]
        f16
    """
    NC, SH, NW, SW = cfg.n_cores, cfg.shard, cfg.nw, cfg.sw
    src = np.asarray(edge_index[0], dtype=np.int64)
    dst = np.asarray(edge_index[1], dtype=np.int64)
    loops = np.arange(cfg.n_nodes, dtype=np.int64)
    src = np.concatenate([src, loops])
    dst = np.concatenate([dst, loops])

    core = dst // SH
    win = (dst - core * SH) // P

    order = np.lexsort((src, win, core))
    src_s, dst_s, core_s, win_s = src[order], dst[order], core[order], win[order]
    key = core_s * NW + win_s
    starts = np.searchsorted(key, np.arange(NC * NW))
    ends = np.searchsorted(key, np.arange(NC * NW) + 1)

    lo_edges = [[None] * NW for _ in range(NC)]
    hi_edges = [[None] * NW for _ in range(NC)]
    for c in range(NC):
        for w in range(NW):
            s, e = starts[c * NW + w], ends[c * NW + w]
            es, ed = src_s[s:e], dst_s[s:e]
            lo = es < cfg.lo_split
            lo_edges[c][w] = (es[lo], ed[lo])
            hi_edges[c][w] = (es[~lo], ed[~lo])

    nch_lo = [0] * NW
    nch_hi = [0] * NW
    for w in range(NW):
        ml = max(len(lo_edges[c][w][0]) for c in range(NC))
        mh = max(len(hi_edges[c][w][0]) for c in range(NC))
        nch_lo[w] = math.ceil(ml / P) if ml else 0
        nch_hi[w] = math.ceil(mh / P) if mh else 0
        if nch_lo[w] == 0 and nch_hi[w] == 0:
            nch_lo[w] = 1  # degenerate empty window: keep shapes legal

    # group windows into superwindows of SW
    groups = []
    for w0 in range(0, NW, SW):
        ws = list(range(w0, min(w0 + SW, NW)))
        nlo = sum(nch_lo[w] for w in ws)
        nhi = sum(nch_hi[w] for w in ws)
        winfo = []
        lo_off = 0
        hi_off = nlo
        for w in ws:
            slots = (list(range(lo_off, lo_off + nch_lo[w]))
                     + list(range(hi_off, hi_off + nch_hi[w])))
            lo_off += nch_lo[w]
            hi_off += nch_hi[w]
            winfo.append(dict(w=w, rows=min(P, SH - w * P), slots=slots))
        groups.append(dict(nlo=nlo, nhi=nhi, nch=nlo + nhi, win=winfo))

    nch_tot = sum(g["nch"] for g in groups)
    ncols = 8 * nch_tot  # idx cols per core: (nch*128)/16

    def wrap16(vals, n_idx):
        """[n_idx] int -> [128, n_idx//16] int16 in dma_gather layout."""
        cols = n_idx // 16
        out = np.zeros((16, cols), dtype=np.int16)
        v = np.asarray(vals, dtype=np.int64)
        out[np.arange(n_idx) % 16, np.arange(n_idx) // 16] = v
        return np.tile(out, (8, 1))

    per_core = []
    for c in range(NC):
        gidx = np.zeros((P, ncols), dtype=np.int16)
        dstoff = np.full((P, nch_tot), 255.0, dtype=np.float16)
        dstrow = np.full((1, nch_tot * P), 255.0, dtype=np.float16)  # replicated below
        gcol = 0
        ccol = 0
        for g in groups:
            ws = [wi["w"] for wi in g["win"]]
            # slot-ordered per-chunk data: lo chunks (window-major) then hi
            gvals = []   # table row index (region-relative)
            avals = []   # local dst row index
            ovals = []   # dst offset within window (or 255 pad)
            for region, base in ((lo_edges, 0), (hi_edges, cfg.lo_split)):
                counts = nch_lo if base == 0 else nch_hi
                for w in ws:
                    nchunks = counts[w]
                    if nchunks == 0:
                        continue
                    es, ed = region[c][w]
                    n_idx = nchunks * P
                    gv = np.zeros(n_idx, dtype=np.int64)
                    av = np.zeros(n_idx, dtype=np.int64)
                    ov = np.full(n_idx, 255.0, dtype=np.float16)
                    k = len(es)
                    gv[:k] = es - base
                    d_local = ed - c * SH
                    av[:k] = d_local
                    ov[:k] = (d_local - w * P).astype(np.float16)
                    gvals.append(gv)
                    avals.append(av)
                    ovals.append(ov)
            gv = np.concatenate(gvals)
            av = np.concatenate(avals)
            ov = np.concatenate(ovals)
            n_idx = len(gv)
            assert n_idx == g["nch"] * P
            gidx[:, gcol:gcol + n_idx // 16] = wrap16(gv, n_idx)
            dstoff[:, ccol:ccol + g["nch"]] = ov.reshape(g["nch"], P).T
            dstrow[0, ccol * P:(ccol + g["nch"]) * P] = ov
            gcol += n_idx // 16
            ccol += g["nch"]
        assert gcol == ncols and ccol == nch_tot
        per_core.append(dict(gidx=gidx, dstoff=dstoff,
                             dstrow=np.tile(dstrow, (P, 1))))

    plan = dict(groups=groups, nch_tot=nch_tot, ncols=ncols)
    return plan, per_core


def _pack_wext(cfg, W, a_src, a_dst):
    """[K, 256] weight -> [K, 384] f16: [W | W@Asrc | W@Adst | 0]."""
    K = W.shape[0]
    H, C = cfg.heads, cfg.hid
    out = np.zeros((K, cfg.ts), dtype=np.float32)
    out[:, :cfg.d1] = W
    for h in range(H):
        out[:, cfg.d1 + h] = W[:, h * C:(h + 1) * C] @ a_src[h]
        out[:, cfg.d1 + 4 + h] = W[:, h * C:(h + 1) * C] @ a_dst[h]
    return out.astype(np.float16)


def _ap(t, offset_elems, free_pattern):
    """SBUF AP with explicit free [step, count] dims on top of a tile AP."""
    import concourse.bass as bass
    return bass.AP(t.tensor, t.offset + offset_elems,
                   [list(t.ap[0])] + [list(p) for p in free_pattern])


def _apd(t, offset_elems, pattern):
    """DRAM AP with fully explicit [step, count] dims (no partition dim)."""
    import concourse.bass as bass
    return bass.AP(t.tensor, t.offset + offset_elems,
                   [list(p) for p in pattern])


def build_program(cfg, plan):
    import concourse.bass as bass
    import concourse.mybir as mybir
    import concourse.tile as tile
    from concourse import bacc
    from concourse.masks import make_identity
    from contextlib import ExitStack

    f32 = mybir.dt.float32
    f16 = mybir.dt.float16
    i16 = mybir.dt.int16
    TS, D1, H, C = cfg.ts, cfg.d1, cfg.heads, cfg.hid
    SH, NW, NC = cfg.shard, cfg.nw, cfg.n_cores
    GROUPS = plan["groups"]
    NCOLS = plan["ncols"]
    LO = cfg.lo_split
    N = cfg.n_nodes
    HI_ROWS = N - LO
    k2_tiles = D1 // P            # 2 for layer 2

    nc = bacc.Bacc(dynamic_dma_scratch_size=32768)

    xT = nc.dram_tensor("xT", [cfg.in_dim, SH], f16, kind="ExternalInput")
    w1e = nc.dram_tensor("w1e", [cfg.in_dim, TS], f16, kind="ExternalInput")
    w2e = nc.dram_tensor("w2e", [D1, TS], f16, kind="ExternalInput")
    gidx_d = nc.dram_tensor("gidx", [P, NCOLS], i16, kind="ExternalInput")
    dstoff_d = nc.dram_tensor("dstoff", [P, plan["nch_tot"]], f16,
                              kind="ExternalInput")
    dstrow_d = nc.dram_tensor("dstrow", [P, plan["nch_tot"] * P], f16,
                              kind="ExternalInput")
    iotaf_d = nc.dram_tensor("iotaf", [P, P], f16, kind="ExternalInput")
    iotac_d = nc.dram_tensor("iotac", [P, 1], f16, kind="ExternalInput")
    out_d = nc.dram_tensor("out", [SH, D1], f32, kind="ExternalOutput")

    with ExitStack() as ctx:
        tc = ctx.enter_context(tile.TileContext(nc))
        const = ctx.enter_context(tc.tile_pool(name="const", bufs=1))
        sb = ctx.enter_context(tc.tile_pool(name="sb", bufs=2))
        eps = ctx.enter_context(tc.tile_pool(name="eps", bufs=2))
        gch = ctx.enter_context(tc.tile_pool(name="gch", bufs=3))
        psum = ctx.enter_context(tc.tile_pool(name="psum", bufs=2, space="PSUM"))
        dram = ctx.enter_context(tc.tile_pool(name="dram", bufs=1, space="DRAM"))

        # ---- constants / static inputs into SBUF
        w1e_sb = const.tile([cfg.in_dim, TS], f16)
        nc.sync.dma_start(out=w1e_sb[:], in_=w1e[:, :])
        w2e_sb = [const.tile([P, TS], f16, tag=f"w2e{k}", name=f"w2e_sb{k}")
                  for k in range(k2_tiles)]
        for k in range(k2_tiles):
            nc.sync.dma_start(out=w2e_sb[k][:], in_=w2e[k * P:(k + 1) * P, :])
        gidx_sb = const.tile([P, NCOLS], i16)
        nc.sync.dma_start(out=gidx_sb[:], in_=gidx_d[:, :])
        iotac_sb = const.tile([P, 1], f16, tag="iotac", name="iotac_sb")
        nc.sync.dma_start(out=iotac_sb[:], in_=iotac_d[:, :])
        aldst_sb = [const.tile([P, NW * 4], f16, tag=f"ald{i}",
                               name=f"aldst_sb{i}") for i in range(2)]
        dstoff_sb = const.tile([P, plan["nch_tot"]], f16)
        nc.sync.dma_start(out=dstoff_sb[:], in_=dstoff_d[:, :])
        iotaf_sb = const.tile([P, P], f16)
        nc.sync.dma_start(out=iotaf_sb[:], in_=iotaf_d[:, :])
        ident = const.tile([P, P], f16)
        make_identity(nc, ident[:])


        t_shard = [dram.tile([SH, TS], f16, tag=f"tsh{i}", name=f"t_shard{i}")
                   for i in range(2)]
        t_full = [dram.tile([N, TS], f16, tag=f"tfu{i}", name=f"t_full{i}",
                            addr_space="Shared") for i in range(2)]
        cgroups = [list(range(NC))]

        # ---- phase 1: layer-1 table for own shard, from xT input
        for w in range(NW):
            rows = min(P, SH - w * P)
            xt = sb.tile([cfg.in_dim, P], f16, tag="xt")
            nc.sync.dma_start(out=xt[:, :rows], in_=xT[:, w * P:w * P + rows])
            ps = psum.tile([P, TS], f32, tag="tps")
            nc.tensor.matmul(out=ps[:rows, :], lhsT=xt[:, :rows], rhs=w1e_sb[:],
                             start=True, stop=True)
            tsb = sb.tile([P, TS], f16, tag="tsb")
            nc.scalar.copy(out=tsb[:rows, :], in_=ps[:rows, :])
            nc.scalar.copy(out=aldst_sb[0][:rows, w * 4:w * 4 + 4],
                           in_=tsb[:rows, D1 + 4:D1 + 8])
            nc.sync.dma_start(out=t_shard[0][w * P:w * P + rows, :],
                              in_=tsb[:rows, :])

        nc.gpsimd.collective_compute(
            "AllGather", mybir.AluOpType.bypass, replica_groups=cgroups,
            ins=[t_shard[0][:, :]], outs=[t_full[0][:, :]])

        # ---- edge phase (shared between the two layers)

        def edge_phase(layer):
            import os
            ngrp = int(os.environ.get("GAT_NGRP", "1000000"))
            parts = os.environ.get("GAT_PARTS", "")
            tf, tsh = t_full[layer], t_shard[layer]
            gcol = 0
            ccol = 0
            for gn, g in enumerate(GROUPS):
                if gn >= ngrp:
                    break
                nch, nlo, nhi = g["nch"], g["nlo"], g["nhi"]
                G = gch.tile([P, nch * TS], f16, tag="G")
                ALd = gch.tile([P, nch * 4], f16, tag="ALd")
                ST = gch.tile([P, nch * P], f16, tag="ST")
                STT = gch.tile([P, nch * P], f16, tag="STT")
                dstrow = gch.tile([P, nch * P], f16, tag="dstrow")
                nc.sync.dma_start(
                    out=dstrow[:],
                    in_=dstrow_d[:, ccol * P:(ccol + nch) * P])

                if "nogather" in parts:
                    nc.vector.memset(G[:], 0.001)
                    nc.vector.memset(ALd[:], 0.001)
                # gathers: src rows (lo/hi) from full table, al rows
                # local.  Real ucode burns ~1 ring descriptor per index
                # (carveout = dynamic_dma_scratch_size/16), so cap each call
                # at MAXCK chunks.
                MAXCK = 8
                for nck, coff, base in (() if "nogather" in parts else
                                        ((nlo, 0, 0), (nhi, nlo, LO))):
                    if nck == 0:
                        continue
                    nrows = min(N, LO) if base == 0 else HI_ROWS
                    for c0 in range(0, nck, MAXCK):
                        cn = min(MAXCK, nck - c0)
                        gc = gcol + 8 * (coff + c0)
                        nc.gpsimd.dma_gather(
                            out_ap=_ap(G[:], (coff + c0) * TS,
                                       [[TS, cn], [1, TS]]),
                            in_ap=_apd(tf[:], base * TS,
                                       [[TS, nrows], [1, TS]]),
                            idxs_ap=gidx_sb[:, gc:gc + 8 * cn],
                            num_idxs=cn * P, num_idxs_reg=cn * P,
                            elem_size=TS, elem_step=TS)

                # one-hot mask ST[e, (chunk), d] = (dstoff == d)
                nc.vector.tensor_tensor(
                    out=_ap(ST[:], 0, [[P, nch], [1, P]]),
                    in0=_ap(dstoff_sb[:], ccol, [[1, nch], [0, P]]),
                    in1=_ap(iotaf_sb[:], 0, [[0, nch], [1, P]]),
                    op=mybir.AluOpType.is_equal)
                # transposed mask STT[d, (chunk), e] = (dstrow == d), then
                # per-edge al_dst = STT_c^T @ aldst_w via PE (replaces the
                # per-edge al gather: descriptors are the bottleneck)
                nc.vector.tensor_tensor(
                    out=_ap(STT[:], 0, [[P, nch], [1, P]]),
                    in0=_ap(dstrow[:], 0, [[P, nch], [1, P]]),
                    in1=_ap(iotac_sb[:], 0, [[0, nch], [0, P]]),
                    op=mybir.AluOpType.is_equal)
                alp = psum.tile([P, nch * 4], f32, tag="alp")
                for wi in g["win"]:
                    for slot in wi["slots"]:
                        nc.tensor.matmul(
                            out=alp[:, slot * 4:(slot + 1) * 4],
                            lhsT=STT[:, slot * P:(slot + 1) * P],
                            rhs=aldst_sb[layer][:, wi["w"] * 4:wi["w"] * 4 + 4],
                            start=True, stop=True)
                nc.scalar.copy(out=ALd[:], in_=alp[:])

                # scores in G[:, D1:D1+4]: ex = exp(leaky_relu(al_src+al_dst))
                if "noscore" not in parts:
                  nc.vector.tensor_tensor(
                    out=_ap(G[:], D1, [[TS, nch], [1, 4]]),
                    in0=_ap(G[:], D1, [[TS, nch], [1, 4]]),
                    in1=_ap(ALd[:], 0, [[4, nch], [1, 4]]),
                    op=mybir.AluOpType.add)
                  nc.vector.scalar_tensor_tensor(
                    out=_ap(G[:], D1, [[TS, nch], [1, 4]]),
                    in0=_ap(G[:], D1, [[TS, nch], [1, 4]]),
                    scalar=0.2,
                    in1=_ap(G[:], D1, [[TS, nch], [1, 4]]),
                    op0=mybir.AluOpType.mult, op1=mybir.AluOpType.max)
                  nc.scalar.activation(
                    out=_ap(G[:], D1, [[TS, nch], [1, 4]]),
                    in_=_ap(G[:], D1, [[TS, nch], [1, 4]]),
                    func=mybir.ActivationFunctionType.Exp)

                # weight gathered h rows by ex (all heads, one 3-dim-AP op)
                if "noex" not in parts:
                  nc.vector.tensor_tensor(
                    out=_ap(G[:], 0, [[TS, nch], [1, D1]]),
                    in0=_ap(G[:], 0, [[TS, nch], [1, D1]]),
                    in1=_ap(G[:], D1, [[TS, nch], [1, H], [0, C]]),
                    op=mybir.AluOpType.mult)

                for wi in g["win"]:
                    w, rows, slots = wi["w"], wi["rows"], wi["slots"]
                    # segment sum: psum[d, 0:260] += ST_c^T @ G_c
                    agg = psum.tile([P, D1 + 4], f32, tag="agg")
                    if "nomm" in parts:
                        nc.vector.memset(agg[:], 0.5)
                    for i, slot in enumerate(() if "nomm" in parts else slots):
                        nc.tensor.matmul(
                            out=agg[:, :],
                            lhsT=ST[:, slot * P:(slot + 1) * P],
                            rhs=G[:, slot * TS:slot * TS + D1 + 4],
                            start=(i == 0), stop=(i == len(slots) - 1))

                    # normalize + relu (+ next-layer table / output write)
                    den = eps.tile([P, 4], f32, tag="den")
                    nc.vector.tensor_scalar_max(out=den[:], in0=agg[:, D1:D1 + 4],
                                                scalar1=1e-30)
                    rec = eps.tile([P, 4], f32, tag="rec")
                    nc.vector.reciprocal(out=rec[:], in_=den[:])

                    if layer == 0:
                        act = eps.tile([P, D1], f16, tag="act")
                        for h in range(H):
                            nc.scalar.activation(
                                out=act[:rows, h * C:(h + 1) * C],
                                in_=agg[:rows, h * C:(h + 1) * C],
                                func=mybir.ActivationFunctionType.Relu,
                                scale=rec[:rows, h:h + 1])
                        # layer-2 table rows: transpose act, matmul with w2e
                        tp = psum.tile([P, D1], f16, tag="tp")
                        xT2 = eps.tile([P, D1], f16, tag="xT2")
                        if "notp" in parts:
                            nc.vector.memset(xT2[:], 0.1)
                        for k in range(() and 0 or (0 if "notp" not in parts else k2_tiles), k2_tiles):
                            nc.tensor.transpose(
                                out=tp[:, k * P:k * P + rows],
                                in_=act[:rows, k * P:(k + 1) * P],
                                identity=ident[:rows, :rows])
                        for k in range(k2_tiles if "notp" in parts else 0, k2_tiles):
                            nc.vector.tensor_copy(
                                out=xT2[:, k * P:k * P + rows],
                                in_=tp[:, k * P:k * P + rows])
                        t2p = psum.tile([P, TS], f32, tag="tps")
                        for k in range(k2_tiles):
                            nc.tensor.matmul(
                                out=t2p[:rows, :],
                                lhsT=xT2[:, k * P:k * P + rows],
                                rhs=w2e_sb[k][:],
                                start=(k == 0), stop=(k == k2_tiles - 1))
                        t2sb = eps.tile([P, TS], f16, tag="t2sb")
                        nc.scalar.copy(out=t2sb[:rows, :], in_=t2p[:rows, :])
                        nc.scalar.copy(out=aldst_sb[1][:rows, w * 4:w * 4 + 4],
                                       in_=t2sb[:rows, D1 + 4:D1 + 8])
                        nc.sync.dma_start(out=t_shard[1][w * P:w * P + rows, :],
                                          in_=t2sb[:rows, :])
                    else:
                        act = eps.tile([P, D1], f32, tag="act")
                        for h in range(H):
                            nc.scalar.activation(
                                out=act[:rows, h * C:(h + 1) * C],
                                in_=agg[:rows, h * C:(h + 1) * C],
                                func=mybir.ActivationFunctionType.Relu,
                                scale=rec[:rows, h:h + 1])
                        nc.sync.dma_start(out=out_d[w * P:w * P + rows, :],
                                          in_=act[:rows, :])

                gcol += 8 * nch
                ccol += nch

        import os
        _skip = os.environ.get("GAT_SKIP", "")
        if "e0" not in _skip:
            edge_phase(0)
        if "ag2" not in _skip:
            nc.gpsimd.collective_compute(
                "AllGather", mybir.AluOpType.bypass, replica_groups=cgroups,
                ins=[t_shard[1][:, :]], outs=[t_full[1][:, :]])
        if "e1" not in _skip:
            edge_phase(1)

    nc.compile()  # Bacc legalization: wait relocation, library loads, ISA bytes
    return nc


def _make_inputs(cfg, plan, per_core, x, W1, a1s, a1d, W2, a2s, a2d):
    iotaf = np.tile(np.arange(P, dtype=np.float16), (P, 1))
    iotac = np.arange(P, dtype=np.float16).reshape(P, 1)
    w1e = _pack_wext(cfg, np.asarray(W1, np.float32), np.asarray(a1s, np.float32),
                     np.asarray(a1d, np.float32))
    w2e = _pack_wext(cfg, np.asarray(W2, np.float32), np.asarray(a2s, np.float32),
                     np.asarray(a2d, np.float32))
    x = np.asarray(x, np.float32)
    in_maps = []
    for c in range(cfg.n_cores):
        xs = x[c * cfg.shard:(c + 1) * cfg.shard].T.astype(np.float16).copy()
        in_maps.append(dict(
            xT=xs, w1e=w1e, w2e=w2e, iotaf=iotaf, iotac=iotac,
            gidx=per_core[c]["gidx"], dstoff=per_core[c]["dstoff"],
            dstrow=per_core[c]["dstrow"]))
    return in_maps


def _ensure_ntff_hook():
    """Register the axon NTFF profiling hook if the antenv shim is absent."""
    import types
    try:
        from antenv.axon_hooks import get_axon_ntff_profile_hook  # noqa: F401
        return
    except ImportError:
        pass
    import antenv
    mod = types.ModuleType("antenv.axon_hooks")
    _h = [None]
    mod.set_axon_ntff_profile_hook = lambda h: _h.__setitem__(0, h)
    mod.get_axon_ntff_profile_hook = lambda: _h[0]
    sys.modules["antenv.axon_hooks"] = mod
    antenv.axon_hooks = mod
    try:
        from trn_agent_boot.trn_boot import _ntff_profile_via_ctypes
        mod.set_axon_ntff_profile_hook(
            _ntff_profile_via_ctypes("/opt/axon/libaxon_pjrt.so"))
    except Exception:
        pass


def run(cfg, inputs, trace=False):
    from concourse.bass_utils import run_bass_kernel_spmd

    if trace:
        _ensure_ntff_hook()

    plan, per_core = _plan_edges(cfg, np.asarray(inputs["edge_index"]))
    nc = build_program(cfg, plan)
    in_maps = _make_inputs(cfg, plan, per_core, inputs["x"],
                           inputs["W1"], inputs["a1_src"], inputs["a1_dst"],
                           inputs["W2"], inputs["a2_src"], inputs["a2_dst"])
    b1 = np.asarray(inputs["b1"], np.float32)
    b2 = np.asarray(inputs["b2"], np.float32)
    assert not (np.any(b1) or np.any(b2)), "nonzero biases not supported"
    res = run_bass_kernel_spmd(nc, in_maps, list(range(cfg.n_cores)),
                               trace=trace)
    out = np.concatenate([res.results[c]["out"] for c in range(cfg.n_cores)],
                         axis=0)
    return out, res


def kernel(**inputs) -> np.ndarray:
    cfg = Cfg()
    assert inputs["x"].shape == (cfg.n_nodes, cfg.in_dim)
    out, _ = run(cfg, inputs, trace=False)
    return out.astype(np.float32)


# revision 26
# speedup vs baseline: 2.7536x; 1.1643x over previous
"""GAT 2-layer encoder kernel for Trainium2 (8 NeuronCores, Bass/Tile).

Strategy (graph/data parallel, dst-sharded), v2:
  - Nodes sharded contiguously across 8 cores (6250 each); each core owns the
    edges whose destination lands in its shard (plus self loops).
  - Per layer, each core computes an fp16 "node table" for its shard:
        row n = [ h(n) (256 f16) | al_src(n) (4) | al_dst(n) (4) | pad to 384 ]
    Tables are AllGather'd so every core has the full [N, 384] f16 table.
  - Edge phase runs on groups of SW destination windows (128 dst each).
    Per group: three ASYNC dma_gather preps (prepare_only=True) — source table
    rows from the lo/hi halves of the full table (int16 idx limit) and the
    al_dst rows from the local shard table — fired by one trigger_dma.  The
    GpSimd engine only pays descriptor generation; transfers overlap compute.
  - Scores ex = exp(leaky_relu(al_src + al_dst)) computed in fp16 on
    vector/scalar; gathered h rows scaled by ex in one 3-dim-AP vector op;
    per-window segment-sum via PE matmuls (fp16 in, fp32 PSUM accumulate):
    onehot(dst)^T @ [ex*h | ex].  Normalization fused into Relu activations.
  - Layer-1 epilogue transposes activations and computes layer-2 table rows;
    layer-2 epilogue writes final f32 output rows.

Edge structure (indices, chunk counts) is baked in at build time; per-window
chunk counts are maxed across cores so one SPMD program runs on all 8 cores
with per-core index data.
"""

import math
import sys

import numpy as np

sys.path.insert(0, "/opt/trn_rl_repo")

P = 128  # partitions


class Cfg:
    def __init__(self, n_nodes=50000, in_dim=128, heads=4, hid=64,
                 n_cores=8, sw=2, splw=None):
        self.n_nodes = n_nodes
        self.in_dim = in_dim
        self.heads = heads
        self.hid = hid
        self.n_cores = n_cores
        self.d1 = heads * hid                       # 256
        self.ts = 384                               # f16 table row stride
        self.sw = sw                                # windows per gather group
        assert n_nodes % n_cores == 0
        self.shard = n_nodes // n_cores             # 6250
        self.nw = math.ceil(self.shard / P)         # windows per core (49)
        self.shard_pad = self.nw * P
        self.lo_split = 32768                       # int16-safe table split
        assert splw is None


def _plan_edges(cfg, edge_index):
    """Host-side: per-core, per-group padded edge lists in gather layout.

    Self loops are NOT added here (handled locally per window on-device).
    Sources are split into region A (shard rows < splr) and region B (rest),
    matching the two AllGather half-tables; indices are region-local so they
    fit int16.  Chunk slot order within a group: all A chunks (window-major),
    then all B chunks.  Per-window chunk counts are maxed across cores so the
    slot structure is SPMD-uniform; indices/offsets are per-core data.
    """
    NC, SH, NW, SW = cfg.n_cores, cfg.shard, cfg.nw, cfg.sw
    LO = cfg.lo_split
    src = np.asarray(edge_index[0], dtype=np.int64)
    dst = np.asarray(edge_index[1], dtype=np.int64)

    core = dst // SH
    win = (dst - core * SH) // P

    order = np.lexsort((src, win, core))
    src_s, dst_s, core_s, win_s = src[order], dst[order], core[order], win[order]
    key = core_s * NW + win_s
    starts = np.searchsorted(key, np.arange(NC * NW))
    ends = np.searchsorted(key, np.arange(NC * NW) + 1)

    a_edges = [[None] * NW for _ in range(NC)]
    b_edges = [[None] * NW for _ in range(NC)]
    for c in range(NC):
        for w in range(NW):
            s, e = starts[c * NW + w], ends[c * NW + w]
            es, ed = src_s[s:e], dst_s[s:e]
            a = es < LO
            a_edges[c][w] = (es[a], ed[a])
            b_edges[c][w] = (es[~a], ed[~a])

    nch_a = [0] * NW
    nch_b = [0] * NW
    for w in range(NW):
        ma = max(len(a_edges[c][w][0]) for c in range(NC))
        mb = max(len(b_edges[c][w][0]) for c in range(NC))
        nch_a[w] = math.ceil(ma / P) if ma else 0
        nch_b[w] = math.ceil(mb / P) if mb else 0
        if nch_a[w] == 0 and nch_b[w] == 0:
            nch_a[w] = 1  # degenerate empty window: keep shapes legal

    # group windows into superwindows of SW
    groups = []
    for w0 in range(0, NW, SW):
        ws = list(range(w0, min(w0 + SW, NW)))
        na = sum(nch_a[w] for w in ws)
        nb = sum(nch_b[w] for w in ws)
        winfo = []
        a_off = 0
        b_off = na
        for w in ws:
            slots = (list(range(a_off, a_off + nch_a[w]))
                     + list(range(b_off, b_off + nch_b[w])))
            a_off += nch_a[w]
            b_off += nch_b[w]
            winfo.append(dict(w=w, rows=min(P, SH - w * P), slots=slots))
        groups.append(dict(nlo=na, nhi=nb, nch=na + nb, win=winfo))

    nch_tot = sum(g["nch"] for g in groups)
    ncols = 8 * nch_tot  # idx cols per core: (nch*128)/16

    def wrap16(vals, n_idx):
        """[n_idx] int -> [128, n_idx//16] int16 in dma_gather layout."""
        cols = n_idx // 16
        out = np.zeros((16, cols), dtype=np.int16)
        v = np.asarray(vals, dtype=np.int64)
        out[np.arange(n_idx) % 16, np.arange(n_idx) // 16] = v
        return np.tile(out, (8, 1))

    per_core = []
    for c in range(NC):
        gidx = np.zeros((P, ncols), dtype=np.int16)
        dstoff = np.full((P, nch_tot), 255.0, dtype=np.float16)
        dstrow = np.full((1, nch_tot * P), 255.0, dtype=np.float16)
        gcol = 0
        ccol = 0
        for g in groups:
            ws = [wi["w"] for wi in g["win"]]
            gvals = []   # region-local table row index
            ovals = []   # dst offset within window (or 255 pad)
            for region, counts, is_b in ((a_edges, nch_a, False),
                                         (b_edges, nch_b, True)):
                for w in ws:
                    nchunks = counts[w]
                    if nchunks == 0:
                        continue
                    es, ed = region[c][w]
                    n_idx = nchunks * P
                    gv = np.zeros(n_idx, dtype=np.int64)
                    ov = np.full(n_idx, 255.0, dtype=np.float16)
                    k = len(es)
                    gv[:k] = es - (LO if is_b else 0)
                    ov[:k] = (ed - c * SH - w * P).astype(np.float16)
                    gvals.append(gv)
                    ovals.append(ov)
            gv = np.concatenate(gvals)
            ov = np.concatenate(ovals)
            n_idx = len(gv)
            assert n_idx == g["nch"] * P
            gidx[:, gcol:gcol + n_idx // 16] = wrap16(gv, n_idx)
            dstoff[:, ccol:ccol + g["nch"]] = ov.reshape(g["nch"], P).T
            dstrow[0, ccol * P:(ccol + g["nch"]) * P] = ov
            gcol += n_idx // 16
            ccol += g["nch"]
        assert gcol == ncols and ccol == nch_tot
        per_core.append(dict(gidx=gidx, dstoff=dstoff,
                             dstrow=np.tile(dstrow, (P, 1))))

    plan = dict(groups=groups, nch_tot=nch_tot, ncols=ncols)
    return plan, per_core


def _pack_wext(cfg, W, a_src, a_dst):
    """[K, 256] weight -> [K, 384] f16: [W | W@Asrc | W@Adst | 0]."""
    K = W.shape[0]
    H, C = cfg.heads, cfg.hid
    out = np.zeros((K, cfg.ts), dtype=np.float32)
    out[:, :cfg.d1] = W
    for h in range(H):
        out[:, cfg.d1 + h] = W[:, h * C:(h + 1) * C] @ a_src[h]
        out[:, cfg.d1 + 4 + h] = W[:, h * C:(h + 1) * C] @ a_dst[h]
    return out.astype(np.float16)


def _ap(t, offset_elems, free_pattern):
    """SBUF AP with explicit free [step, count] dims on top of a tile AP."""
    import concourse.bass as bass
    return bass.AP(t.tensor, t.offset + offset_elems,
                   [list(t.ap[0])] + [list(p) for p in free_pattern])


def _apd(t, offset_elems, pattern):
    """DRAM AP with fully explicit [step, count] dims (no partition dim)."""
    import concourse.bass as bass
    return bass.AP(t.tensor, t.offset + offset_elems,
                   [list(p) for p in pattern])


def build_program(cfg, plan):
    import concourse.bass as bass
    import concourse.mybir as mybir
    import concourse.tile as tile
    from concourse import bacc
    from concourse.masks import make_identity
    from contextlib import ExitStack

    f32 = mybir.dt.float32
    f16 = mybir.dt.float16
    i16 = mybir.dt.int16
    TS, D1, H, C = cfg.ts, cfg.d1, cfg.heads, cfg.hid
    SH, NW, NC = cfg.shard, cfg.nw, cfg.n_cores
    GROUPS = plan["groups"]
    NCOLS = plan["ncols"]
    N = cfg.n_nodes
    LO = cfg.lo_split
    A_ROWS = min(N, LO)           # region-A table rows
    B_ROWS = N - LO               # region-B table rows
    k2_tiles = D1 // P            # 2 for layer 2

    nc = bacc.Bacc(dynamic_dma_scratch_size=32768)

    xT = nc.dram_tensor("xT", [cfg.in_dim, SH], f16, kind="ExternalInput")
    w1e = nc.dram_tensor("w1e", [cfg.in_dim, TS], f16, kind="ExternalInput")
    w2e = nc.dram_tensor("w2e", [D1, TS], f16, kind="ExternalInput")
    gidx_d = nc.dram_tensor("gidx", [P, NCOLS], i16, kind="ExternalInput")
    dstoff_d = nc.dram_tensor("dstoff", [P, plan["nch_tot"]], f16,
                              kind="ExternalInput")
    dstrow_d = nc.dram_tensor("dstrow", [P, plan["nch_tot"] * P], f16,
                              kind="ExternalInput")
    iotaf_d = nc.dram_tensor("iotaf", [P, P], f16, kind="ExternalInput")
    iotac_d = nc.dram_tensor("iotac", [P, 1], f16, kind="ExternalInput")
    out_d = nc.dram_tensor("out", [SH, D1], f32, kind="ExternalOutput")

    with ExitStack() as ctx:
        tc = ctx.enter_context(tile.TileContext(nc))
        const = ctx.enter_context(tc.tile_pool(name="const", bufs=1))
        sb = ctx.enter_context(tc.tile_pool(name="sb", bufs=2))
        eps = ctx.enter_context(tc.tile_pool(name="eps", bufs=2))
        gch = ctx.enter_context(tc.tile_pool(name="gch", bufs=3))
        psum = ctx.enter_context(tc.tile_pool(name="psum", bufs=2, space="PSUM"))
        dram = ctx.enter_context(tc.tile_pool(name="dram", bufs=1, space="DRAM"))

        # ---- constants / static inputs into SBUF
        w1e_sb = const.tile([cfg.in_dim, TS], f16)
        nc.sync.dma_start(out=w1e_sb[:], in_=w1e[:, :])
        w2e_sb = [const.tile([P, TS], f16, tag=f"w2e{k}", name=f"w2e_sb{k}")
                  for k in range(k2_tiles)]
        for k in range(k2_tiles):
            nc.sync.dma_start(out=w2e_sb[k][:], in_=w2e[k * P:(k + 1) * P, :])
        gidx_sb = const.tile([P, NCOLS], i16)
        nc.sync.dma_start(out=gidx_sb[:], in_=gidx_d[:, :])
        iotac_sb = const.tile([P, 1], f16, tag="iotac", name="iotac_sb")
        nc.sync.dma_start(out=iotac_sb[:], in_=iotac_d[:, :])
        aldst_sb = [const.tile([P, NW * 4], f16, tag=f"ald{i}",
                               name=f"aldst_sb{i}") for i in range(2)]
        dstoff_sb = const.tile([P, plan["nch_tot"]], f16)
        nc.sync.dma_start(out=dstoff_sb[:], in_=dstoff_d[:, :])
        iotaf_sb = const.tile([P, P], f16)
        nc.sync.dma_start(out=iotaf_sb[:], in_=iotaf_d[:, :])
        ident = const.tile([P, P], f16)
        make_identity(nc, ident[:])


        t_shard = [dram.tile([SH, TS], f16, tag=f"tsh{i}", name=f"t_shard{i}")
                   for i in range(2)]
        t_full = [dram.tile([N, TS], f16, tag=f"tfu{i}", name=f"t_full{i}",
                            addr_space="Shared") for i in range(2)]
        cgroups = [list(range(NC))]

        def ag_full(layer):
            nc.gpsimd.collective_compute(
                "AllGather", mybir.AluOpType.bypass, replica_groups=cgroups,
                ins=[t_shard[layer][:, :]], outs=[t_full[layer][:, :]])

        # ---- phase 1: layer-1 table for own shard, from xT input
        import os as _os
        _parts0 = _os.environ.get("GAT_PARTS", "")
        for w in range(NW):
            rows = min(P, SH - w * P)
            xt = sb.tile([cfg.in_dim, P], f16, tag="xt")
            nc.sync.dma_start(out=xt[:, :rows], in_=xT[:, w * P:w * P + rows])
            ps = psum.tile([P, TS], f32, tag="tps")
            nc.tensor.matmul(out=ps[:rows, :], lhsT=xt[:, :rows], rhs=w1e_sb[:],
                             start=True, stop=True)
            tsb = sb.tile([P, TS], f16, tag="tsb")
            nc.scalar.copy(out=tsb[:rows, :], in_=ps[:rows, :])
            nc.scalar.copy(out=aldst_sb[0][:rows, w * 4:w * 4 + 4],
                           in_=tsb[:rows, D1 + 4:D1 + 8])
            nc.sync.dma_start(out=t_shard[0][w * P:w * P + rows, :],
                              in_=tsb[:rows, :])
        ag_full(0)

        # ---- edge phase (shared between the two layers)

        def edge_phase(layer):
            import os
            ngrp = int(os.environ.get("GAT_NGRP", "1000000"))
            parts = os.environ.get("GAT_PARTS", "")
            tsh = t_shard[layer]
            gcol = 0
            ccol = 0
            w_done = 0
            for gn, g in enumerate(GROUPS):
                if gn >= ngrp:
                    break
                nch, nlo, nhi = g["nch"], g["nlo"], g["nhi"]
                G = gch.tile([P, nch * TS], f16, tag="G")
                ALd = gch.tile([P, nch * 4], f16, tag="ALd")
                ST = gch.tile([P, nch * P], f16, tag="ST")
                STT = gch.tile([P, nch * P], f16, tag="STT")
                dstrow = gch.tile([P, nch * P], f16, tag="dstrow")
                nc.sync.dma_start(
                    out=dstrow[:],
                    in_=dstrow_d[:, ccol * P:(ccol + nch) * P])

                if "nogather" in parts:
                    nc.vector.memset(G[:], 0.001)
                    nc.vector.memset(ALd[:], 0.001)
                # gathers: src rows (lo/hi) from full table, al rows
                # local.  Real ucode burns ~1 ring descriptor per index
                # (carveout = dynamic_dma_scratch_size/16), so cap each call
                # at MAXCK chunks.
                MAXCK = 8
                for nck, coff, base, nrows in (
                        () if "nogather" in parts else
                        ((nlo, 0, 0, A_ROWS), (nhi, nlo, LO, B_ROWS))):
                    if nck == 0:
                        continue
                    for c0 in range(0, nck, MAXCK):
                        cn = min(MAXCK, nck - c0)
                        gc = gcol + 8 * (coff + c0)
                        nc.gpsimd.dma_gather(
                            out_ap=_ap(G[:], (coff + c0) * TS,
                                       [[TS, cn], [1, TS]]),
                            in_ap=_apd(t_full[layer][:], base * TS,
                                       [[TS, nrows], [1, TS]]),
                            idxs_ap=gidx_sb[:, gc:gc + 8 * cn],
                            num_idxs=cn * P, num_idxs_reg=cn * P,
                            elem_size=TS, elem_step=TS)

                # one-hot mask ST[e, (chunk), d] = (dstoff == d)
                nc.vector.tensor_tensor(
                    out=_ap(ST[:], 0, [[P, nch], [1, P]]),
                    in0=_ap(dstoff_sb[:], ccol, [[1, nch], [0, P]]),
                    in1=_ap(iotaf_sb[:], 0, [[0, nch], [1, P]]),
                    op=mybir.AluOpType.is_equal)
                # transposed mask STT[d, (chunk), e] = (dstrow == d), then
                # per-edge al_dst = STT_c^T @ aldst_w via PE (replaces the
                # per-edge al gather: descriptors are the bottleneck)
                nc.vector.tensor_tensor(
                    out=_ap(STT[:], 0, [[P, nch], [1, P]]),
                    in0=_ap(dstrow[:], 0, [[P, nch], [1, P]]),
                    in1=_ap(iotac_sb[:], 0, [[0, nch], [0, P]]),
                    op=mybir.AluOpType.is_equal)
                alp = psum.tile([P, nch * 4], f32, tag="alp")
                for wi in g["win"]:
                    for slot in wi["slots"]:
                        nc.tensor.matmul(
                            out=alp[:, slot * 4:(slot + 1) * 4],
                            lhsT=STT[:, slot * P:(slot + 1) * P],
                            rhs=aldst_sb[layer][:, wi["w"] * 4:wi["w"] * 4 + 4],
                            start=True, stop=True)
                nc.scalar.copy(out=ALd[:], in_=alp[:])

                # self loops handled locally: own table rows, ex from the
                # node's own al_src+al_dst, aggregated via an identity matmul
                nwin = len(g["win"])
                L = gch.tile([P, nwin * TS], f16, tag="L")
                for k, wi in enumerate(g["win"]):
                    nc.sync.dma_start(
                        out=L[:wi["rows"], k * TS:k * TS + TS],
                        in_=tsh[wi["w"] * P:wi["w"] * P + wi["rows"], :])
                nc.vector.tensor_tensor(
                    out=_ap(L[:], D1, [[TS, nwin], [1, 4]]),
                    in0=_ap(L[:], D1, [[TS, nwin], [1, 4]]),
                    in1=_ap(L[:], D1 + 4, [[TS, nwin], [1, 4]]),
                    op=mybir.AluOpType.add)
                nc.vector.scalar_tensor_tensor(
                    out=_ap(L[:], D1, [[TS, nwin], [1, 4]]),
                    in0=_ap(L[:], D1, [[TS, nwin], [1, 4]]),
                    scalar=0.2,
                    in1=_ap(L[:], D1, [[TS, nwin], [1, 4]]),
                    op0=mybir.AluOpType.mult, op1=mybir.AluOpType.max)
                nc.scalar.activation(
                    out=_ap(L[:], D1, [[TS, nwin], [1, 4]]),
                    in_=_ap(L[:], D1, [[TS, nwin], [1, 4]]),
                    func=mybir.ActivationFunctionType.Exp)
                nc.vector.tensor_tensor(
                    out=_ap(L[:], 0, [[TS, nwin], [1, D1]]),
                    in0=_ap(L[:], 0, [[TS, nwin], [1, D1]]),
                    in1=_ap(L[:], D1, [[TS, nwin], [1, H], [0, C]]),
                    op=mybir.AluOpType.mult)

                # scores in G[:, D1:D1+4]: ex = exp(leaky_relu(al_src+al_dst))
                if "noscore" not in parts:
                  nc.vector.tensor_tensor(
                    out=_ap(G[:], D1, [[TS, nch], [1, 4]]),
                    in0=_ap(G[:], D1, [[TS, nch], [1, 4]]),
                    in1=_ap(ALd[:], 0, [[4, nch], [1, 4]]),
                    op=mybir.AluOpType.add)
                  nc.vector.scalar_tensor_tensor(
                    out=_ap(G[:], D1, [[TS, nch], [1, 4]]),
                    in0=_ap(G[:], D1, [[TS, nch], [1, 4]]),
                    scalar=0.2,
                    in1=_ap(G[:], D1, [[TS, nch], [1, 4]]),
                    op0=mybir.AluOpType.mult, op1=mybir.AluOpType.max)
                  nc.scalar.activation(
                    out=_ap(G[:], D1, [[TS, nch], [1, 4]]),
                    in_=_ap(G[:], D1, [[TS, nch], [1, 4]]),
                    func=mybir.ActivationFunctionType.Exp)

                # weight gathered h rows by ex (all heads, one 3-dim-AP op)
                if "noex" not in parts:
                  nc.vector.tensor_tensor(
                    out=_ap(G[:], 0, [[TS, nch], [1, D1]]),
                    in0=_ap(G[:], 0, [[TS, nch], [1, D1]]),
                    in1=_ap(G[:], D1, [[TS, nch], [1, H], [0, C]]),
                    op=mybir.AluOpType.mult)

                for k, wi in enumerate(g["win"]):
                    w, rows, slots = wi["w"], wi["rows"], wi["slots"]
                    # segment sum: psum[d, 0:260] += ST_c^T @ G_c (+ self)
                    agg = psum.tile([P, D1 + 4], f32, tag="agg")
                    if "nomm" in parts:
                        nc.vector.memset(agg[:], 0.5)
                    for i, slot in enumerate(() if "nomm" in parts else slots):
                        nc.tensor.matmul(
                            out=agg[:, :],
                            lhsT=ST[:, slot * P:(slot + 1) * P],
                            rhs=G[:, slot * TS:slot * TS + D1 + 4],
                            start=(i == 0), stop=False)
                    if "nomm" not in parts:
                        nc.tensor.matmul(
                            out=agg[:, :],
                            lhsT=ident[:rows, :],
                            rhs=L[:rows, k * TS:k * TS + D1 + 4],
                            start=False, stop=True)

                    # normalize + relu (+ next-layer table / output write)
                    den = eps.tile([P, 4], f32, tag="den")
                    nc.vector.tensor_scalar_max(out=den[:], in0=agg[:, D1:D1 + 4],
                                                scalar1=1e-30)
                    rec = eps.tile([P, 4], f32, tag="rec")
                    nc.vector.reciprocal(out=rec[:], in_=den[:])

                    if layer == 0:
                        act = eps.tile([P, D1], f16, tag="act")
                        for h in range(H):
                            nc.scalar.activation(
                                out=act[:rows, h * C:(h + 1) * C],
                                in_=agg[:rows, h * C:(h + 1) * C],
                                func=mybir.ActivationFunctionType.Relu,
                                scale=rec[:rows, h:h + 1])
                        # layer-2 table rows: transpose act, matmul with w2e
                        tp = psum.tile([P, D1], f16, tag="tp")
                        xT2 = eps.tile([P, D1], f16, tag="xT2")
                        if "notp" in parts:
                            nc.vector.memset(xT2[:], 0.1)
                        for k in range(() and 0 or (0 if "notp" not in parts else k2_tiles), k2_tiles):
                            nc.tensor.transpose(
                                out=tp[:, k * P:k * P + rows],
                                in_=act[:rows, k * P:(k + 1) * P],
                                identity=ident[:rows, :rows])
                        for k in range(k2_tiles if "notp" in parts else 0, k2_tiles):
                            nc.vector.tensor_copy(
                                out=xT2[:, k * P:k * P + rows],
                                in_=tp[:, k * P:k * P + rows])
                        t2p = psum.tile([P, TS], f32, tag="tps")
                        for k in range(k2_tiles):
                            nc.tensor.matmul(
                                out=t2p[:rows, :],
                                lhsT=xT2[:, k * P:k * P + rows],
                                rhs=w2e_sb[k][:],
                                start=(k == 0), stop=(k == k2_tiles - 1))
                        t2sb = eps.tile([P, TS], f16, tag="t2sb")
                        nc.scalar.copy(out=t2sb[:rows, :], in_=t2p[:rows, :])
                        nc.scalar.copy(out=aldst_sb[1][:rows, w * 4:w * 4 + 4],
                                       in_=t2sb[:rows, D1 + 4:D1 + 8])
                        nc.sync.dma_start(out=t_shard[1][w * P:w * P + rows, :],
                                          in_=t2sb[:rows, :])
                    else:
                        act = eps.tile([P, D1], f32, tag="act")
                        for h in range(H):
                            nc.scalar.activation(
                                out=act[:rows, h * C:(h + 1) * C],
                                in_=agg[:rows, h * C:(h + 1) * C],
                                func=mybir.ActivationFunctionType.Relu,
                                scale=rec[:rows, h:h + 1])
                        nc.sync.dma_start(out=out_d[w * P:w * P + rows, :],
                                          in_=act[:rows, :])

                gcol += 8 * nch
                ccol += nch

        import os
        _skip = os.environ.get("GAT_SKIP", "")
        if "e0" not in _skip:
            edge_phase(0)
        if "ag2" not in _skip:
            ag_full(1)
        if "e1" not in _skip:
            edge_phase(1)

    nc.compile()  # Bacc legalization: wait relocation, library loads, ISA bytes
    return nc


def _make_inputs(cfg, plan, per_core, x, W1, a1s, a1d, W2, a2s, a2d):
    iotaf = np.tile(np.arange(P, dtype=np.float16), (P, 1))
    iotac = np.arange(P, dtype=np.float16).reshape(P, 1)
    w1e = _pack_wext(cfg, np.asarray(W1, np.float32), np.asarray(a1s, np.float32),
                     np.asarray(a1d, np.float32))
    w2e = _pack_wext(cfg, np.asarray(W2, np.float32), np.asarray(a2s, np.float32),
                     np.asarray(a2d, np.float32))
    x = np.asarray(x, np.float32)
    in_maps = []
    for c in range(cfg.n_cores):
        xs = x[c * cfg.shard:(c + 1) * cfg.shard].T.astype(np.float16).copy()
        in_maps.append(dict(
            xT=xs, w1e=w1e, w2e=w2e, iotaf=iotaf, iotac=iotac,
            gidx=per_core[c]["gidx"], dstoff=per_core[c]["dstoff"],
            dstrow=per_core[c]["dstrow"]))
    return in_maps


def _ensure_ntff_hook():
    """Register the axon NTFF profiling hook if the antenv shim is absent."""
    import types
    try:
        from antenv.axon_hooks import get_axon_ntff_profile_hook  # noqa: F401
        return
    except ImportError:
        pass
    import antenv
    mod = types.ModuleType("antenv.axon_hooks")
    _h = [None]
    mod.set_axon_ntff_profile_hook = lambda h: _h.__setitem__(0, h)
    mod.get_axon_ntff_profile_hook = lambda: _h[0]
    sys.modules["antenv.axon_hooks"] = mod
    antenv.axon_hooks = mod
    try:
        from trn_agent_boot.trn_boot import _ntff_profile_via_ctypes
        mod.set_axon_ntff_profile_hook(
            _ntff_profile_via_ctypes("/opt/axon/libaxon_pjrt.so"))
    except Exception:
        pass


def run(cfg, inputs, trace=False):
    from concourse.bass_utils import run_bass_kernel_spmd

    if trace:
        _ensure_ntff_hook()

    plan, per_core = _plan_edges(cfg, np.asarray(inputs["edge_index"]))
    nc = build_program(cfg, plan)
    in_maps = _make_inputs(cfg, plan, per_core, inputs["x"],
                           inputs["W1"], inputs["a1_src"], inputs["a1_dst"],
                           inputs["W2"], inputs["a2_src"], inputs["a2_dst"])
    b1 = np.asarray(inputs["b1"], np.float32)
    b2 = np.asarray(inputs["b2"], np.float32)
    assert not (np.any(b1) or np.any(b2)), "nonzero biases not supported"
    res = run_bass_kernel_spmd(nc, in_maps, list(range(cfg.n_cores)),
                               trace=trace)
    out = np.concatenate([res.results[c]["out"] for c in range(cfg.n_cores)],
                         axis=0)
    return out, res


def kernel(**inputs) -> np.ndarray:
    cfg = Cfg()
    assert inputs["x"].shape == (cfg.n_nodes, cfg.in_dim)
    out, _ = run(cfg, inputs, trace=False)
    return out.astype(np.float32)
